# revision 67
# baseline (speedup 1.0000x reference)
"""Trainium2 Bass kernel for nn_BertSelfAttention_7962869367489.

Dual-branch (self + cross/"knowledge") BERT attention, B=4, S=1024, K=512,
H=1024, NH=16, HD=64, fp32.

Sharding: 8 cores = (batch b in 0..3) x (head-group hg in 0..1, 8 heads each).
All six projections are column-split by head-group; per-head attention is
entirely core-local; output columns are disjoint per core, so the gather is a
pure concatenation (no collectives).

Per-core pipeline:
  - All six projections run as fp8e4m3 DoubleRow matmuls (0.5 PE rows per
    output element, 2 contraction chunks per instruction) using a hi/lo
    residual split prepared on the host: X_hi=fp8(X), X_lo=fp8(X-X_hi) (raw,
    using fp8 subnormals), W'=32*W, W_hi=fp8(W'), W_lo=fp8(W'-W_hi). The
    three product sets hi*hi + lo_x*hi + hi*lo_w share one product scale (32)
    and accumulate in a single PSUM group; measured accuracy matches bf16.
    The x32 descale folds into the exp scale (QK side, /1024) and into the
    V bias-add multiply (1/32), so no extra DVE work.
  - Projections: QT/KT/KQT/KKT = W.T @ srcT in transposed orientation (bf16
    outs, carrying the x32 scale); Vaug/KVaug = srcT.T @ Wv in normal
    orientation with an augmented column of 2.0, so the ctx matmul also
    produces 2*softmax-denominator, folding the (ctx+kctx)*0.5 branch average
    into the normalization.
  - Per head h: scoresT[t,s] = K_h @ Q_h^T (bf16, contraction HD=64); exp on
    ACT with per-partition mask bias and 1/(8*1024) scale, written bf16;
    after the branch's exps, ctx[s,d|den] accumulates in PSUM in NORMAL
    orientation via lhsT = e-chunk [t,128s], rhs = Vaug_h [t,65] (bf16).
    Accumulation groups run sc-outer/kt-inner so each PSUM region hosts one
    group at a time (interleaved groups in one bank clobber each other).
  - Normalization + branch merge on DVE straight out of PSUM; output DMA'd
    in four head-pair quarters.
  - Remaining projections are split into ~0.64us (jt, sc) units and woven
    between attention branches, paced by PE time (~1420 cycles of filler
    per exp step) so the PE never outruns the ACT exp cadence; knowledge
    branch h runs right after self branch h, with the last four branches
    ordered s6 s7 k6 k7 (knl branches have the smaller exp deficit).
  - Unit order inside each weave gap follows the DMA arrival order: the
    weave is a static PE instruction order, so a unit emitted before its
    weights land head-of-line-blocks everything behind it. DMA chunks are
    ~0.25-0.5MB (smaller is HWDGE-issue-bound at ~650ns/DMA); all small
    constants ship as one host-packed array.
"""
import numpy as np
import ml_dtypes
from contextlib import ExitStack

import concourse.bacc as bacc
import concourse.tile as tile
import concourse.mybir as mybir
from concourse.bass_utils import run_bass_kernel_spmd

F32 = mybir.dt.float32
BF16 = mybir.dt.bfloat16
F8 = mybir.dt.float8e4
AF = mybir.ActivationFunctionType
ALU = mybir.AluOpType
DR = mybir.MatmulPerfMode.DoubleRow

P = 128
S = 1024        # query length
TKS = 1024      # self-branch key length
TKK = 512       # knowledge-branch key length
H = 1024        # model dim (projection contraction)
HG = 512        # per-core output width (8 heads x 64)
NHL = 8         # heads per core
HD = 64
HC = H // P     # 8 contraction chunks
WS = 32.0       # host-side weight prescale (fp8 range centering)
INV = 0.125     # 1/sqrt(64)
INVS = INV / (WS * WS)  # exp scale absorbing the x32 on both Q and K

_CACHE = {}


def _build():
    nc = bacc.Bacc(target_bir_lowering=False, debug=False)

    hsT_hi = nc.dram_tensor("hsT_hi", [H, S], F8, kind="ExternalInput")
    hsT_lo = nc.dram_tensor("hsT_lo", [H, S], F8, kind="ExternalInput")
    ehsT_hi = nc.dram_tensor("ehsT_hi", [H, TKK], F8, kind="ExternalInput")
    ehsT_lo = nc.dram_tensor("ehsT_lo", [H, TKK], F8, kind="ExternalInput")
    # host-packed [wq_hi jt0 | wk_hi jt0 | wq_lo jt0 | wk_lo jt0]: contiguous
    # 512B rows dodge the sub-512B-run DMA penalty on the startup-critical
    # first weight loads
    wqk0 = nc.dram_tensor("wqk0", [H, 4 * P], F8, kind="ExternalInput")
    # host-packed [wq_hi jt1-3 | wk_hi jt1-3 | wq_lo jt1-3 | wk_lo jt1-3]:
    # 1536B rows, one clean DMA for the rest of the q/k weights
    wqkr = nc.dram_tensor("wqkr", [H, 4 * 3 * P], F8, kind="ExternalInput")
    w_hi = {}
    w_lo = {}
    b_in = {}
    for nm in ["v", "kq", "kk", "kv"]:
        w_hi[nm] = nc.dram_tensor(f"w{nm}_hi", [H, HG], F8,
                                  kind="ExternalInput")
        w_lo[nm] = nc.dram_tensor(f"w{nm}_lo", [H, HG], F8,
                                  kind="ExternalInput")
    # host-packed small constants, one DMA slot: [bias_q(4) | bias_k(4) |
    # mask(8) | emask(4) | bias_kq(4) | bias_kk(4)] in (p, col) layout
    c0_in = nc.dram_tensor("c0", [P, 28], F32, kind="ExternalInput")
    # host-packed broadcast rows: [bias_v(512) | bias_kv(512)]
    c1_in = nc.dram_tensor("c1", [P, 2 * HG], F32, kind="ExternalInput")
    out = nc.dram_tensor("out", [S, HG], F32, kind="ExternalOutput")

    with tile.TileContext(nc) as tc, ExitStack() as ctx:
        const = ctx.enter_context(tc.tile_pool(name="const", bufs=1))
        persist = ctx.enter_context(tc.tile_pool(name="persist", bufs=1))
        epool = ctx.enter_context(tc.tile_pool(name="epool", bufs=17))
        smallp = ctx.enter_context(tc.tile_pool(name="smallp", bufs=2))
        snpool = ctx.enter_context(tc.tile_pool(name="snpool", bufs=3))
        psproj = ctx.enter_context(tc.tile_pool(name="psproj", bufs=2, space="PSUM"))
        psbig = ctx.enter_context(tc.tile_pool(name="psbig", bufs=2, space="PSUM"))
        psctx = ctx.enter_context(tc.tile_pool(name="psctx", bufs=2, space="PSUM"))

        # ---- constants (views into the two host-packed tiles) ----
        c0_sb = const.tile([P, 28], F32)
        c1_sb = const.tile([P, 2 * HG], F32)
        bias_col = {"q": c0_sb[:, 0:4], "k": c0_sb[:, 4:8],
                    "kq": c0_sb[:, 20:24], "kk": c0_sb[:, 24:28]}
        mask_sb = c0_sb[:, 8:16]
        emask_sb = c0_sb[:, 16:20]
        bias_row = {"v": c1_sb[:, 0:HG], "kv": c1_sb[:, HG:2 * HG]}
        twos = const.tile([P, 1], F32)
        nc.vector.memset(twos, 2.0)

        def load_consts_early():
            nc.sync.dma_start(out=c0_sb, in_=c0_in.ap())

        def load_consts_late():
            nc.sync.dma_start(out=c1_sb, in_=c1_in.ap())

        # ---- persistent activations ----
        QT = persist.tile([P, 4, S], BF16)       # [j%128, jt, s] (x32 scaled)
        KT = persist.tile([P, 4, TKS], BF16)
        KQT = persist.tile([P, 4, S], BF16)
        KKT = persist.tile([P, 4, TKK], BF16)
        Vaug = persist.tile([P, TKS // P, NHL, HD + 1], BF16)   # [t%128, tt, h, d|2]
        KVaug = persist.tile([P, TKK // P, NHL, HD + 1], BF16)
        hsT_sb = {0: persist.tile([P, HC, S], F8, name="hsT_hi_sb"),
                  1: persist.tile([P, HC, S], F8, name="hsT_lo_sb")}
        ehsT_sb = {0: persist.tile([P, HC, TKK], F8, name="ehsT_hi_sb"),
                   1: persist.tile([P, HC, TKK], F8, name="ehsT_lo_sb")}
        wsb_hi = {}
        wsb_lo = {}
        for nm in ["v", "kq", "kk", "kv"]:
            wsb_hi[nm] = persist.tile([P, HC, HG], F8, name=f"wh_{nm}")
            wsb_lo[nm] = persist.tile([P, HC, HG], F8, name=f"wl_{nm}")
        wqk0_sb = persist.tile([P, HC, 4 * P], F8)
        wqkr_sb = persist.tile([P, HC, 4 * 3 * P], F8)
        # output staging in two head-halves; DMA'd in four head-pair quarters
        out_half = [persist.tile([P, S // P, 4, HD], F32, name=f"out_half{i}",
                                 tag=f"out_half{i}") for i in range(2)]

        # q/k weight operand lookup: (tile, hi col base, lo col base) per jt.
        # wqkr is packed per-jt: [jt block][qh|kh|ql|kl] so each jt's weights
        # load as one contiguous-row DMA in deadline order.
        def qk_cols(nm, jt):
            off = 0 if nm == "q" else 1
            if jt == 0:
                return wqk0_sb, off * P, (2 + off) * P
            base = (jt - 1) * 4 * P
            return wqkr_sb, base + off * P, base + (2 + off) * P

        # ---- input DMAs (sync/HWDGE queue), ordered so the prelude's
        # dependencies (wqk0, hsT_hi, then hsT_lo) land first ----
        def load_rows(dst, src, half, rows, cols):
            nc.sync.dma_start(
                out=dst[:, half * (rows // 2):(half + 1) * (rows // 2), :],
                in_=src[half * (rows * P // 2):(half + 1) * (rows * P // 2), :]
                .rearrange("(hc p) s -> p hc s", p=P))

        def load_w(nm, which):
            src = w_hi[nm] if which == 0 else w_lo[nm]
            dst = wsb_hi[nm] if which == 0 else wsb_lo[nm]
            nc.sync.dma_start(
                out=dst, in_=src.ap().rearrange("(hc p) j -> p hc j", p=P))

        def load_wqk0(hc0, hcn):
            nc.sync.dma_start(
                out=wqk0_sb[:, hc0:hc0 + hcn, :],
                in_=wqk0[hc0 * P:(hc0 + hcn) * P, :].rearrange(
                    "(hc p) j -> p hc j", p=P))

        def load_hsT(which, hc0, hcn):
            src = hsT_hi if which == 0 else hsT_lo
            nc.sync.dma_start(
                out=hsT_sb[which][:, hc0:hc0 + hcn, :],
                in_=src[hc0 * P:(hc0 + hcn) * P, :].rearrange(
                    "(hc p) s -> p hc s", p=P))

        # startup-critical loads in strict first-use order: the prelude's
        # set-A matmuls need wqk0(hi cols) + hsT_hi (first two chunks feed
        # single-hc fp8 matmuls so the PE starts before the pair lands);
        # set-C needs wqk0(lo cols); set-B needs hsT_lo. Then weights in
        # weave-consumption order: wv (G0's V units), wkq/wkk+ehsT (knl0),
        # wkv (kv units), wqkr (q/k jt1-3).
        # 2-hc (0.25MB) chunks: smaller chunks are HWDGE-issue-bound (625ns
        # hold per DMA), larger ones delay the first matmul
        load_wqk0(0, 2)
        load_hsT(0, 0, 2)
        for hc2 in (2, 4, 6):
            load_wqk0(hc2, 2)
            load_hsT(0, hc2, 2)
        load_consts_early()
        # hsT_lo in fine chunks: the prelude's set-B hc-pairs stream as they
        # land instead of waiting for one big transfer
        load_hsT(1, 0, 2)
        load_hsT(1, 2, 2)
        load_hsT(1, 4, 2)
        load_hsT(1, 6, 2)
        load_w("v", 0)
        load_w("v", 1)
        load_consts_late()
        load_w("kq", 0)
        load_w("kq", 1)
        load_rows(ehsT_sb[0], ehsT_hi, 0, HC, TKK)
        load_rows(ehsT_sb[0], ehsT_hi, 1, HC, TKK)
        load_w("kk", 0)
        load_w("kk", 1)
        load_rows(ehsT_sb[1], ehsT_lo, 0, HC, TKK)
        load_rows(ehsT_sb[1], ehsT_lo, 1, HC, TKK)
        load_w("kv", 0)
        load_w("kv", 1)
        for jt in range(3):
            nc.sync.dma_start(
                out=wqkr_sb[:, :, jt * 4 * P:(jt + 1) * 4 * P],
                in_=wqkr[:, jt * 4 * P:(jt + 1) * 4 * P].rearrange(
                    "(hc p) j -> p hc j", p=P))

        # ---- projection emitters (fp8 hi/lo DoubleRow, single PSUM group:
        # sets (w_hi,x_hi), (w_hi,x_lo), (w_lo,x_hi), 4 hc-pairs each) ----
        def proj_t_unit(nm, dst, srcT, jt, sc, w=256):
            if nm in ("q", "k"):
                wt_h, ch, cl = qk_cols(nm, jt)
                wt_l = wt_h
            else:
                wt_h, wt_l = wsb_hi[nm], wsb_lo[nm]
                ch = cl = jt * P
            ps = psproj.tile([P, 512], F32, name="psj", tag="psj")
            mm = 0
            for wt, c0, xs in ((wt_h, ch, 0), (wt_h, ch, 1), (wt_l, cl, 0)):
                for hp in range(4):
                    nc.tensor.matmul(
                        ps[:, 0:w],
                        lhsT=wt[:, 2 * hp:2 * hp + 2, c0:c0 + P],
                        rhs=srcT[xs][:, 2 * hp:2 * hp + 2,
                                     sc * w:(sc + 1) * w],
                        start=(mm == 0), stop=(mm == 11), perf_mode=DR)
                    mm += 1
            nc.vector.tensor_scalar_add(
                dst[:, jt, sc * w:(sc + 1) * w], ps[:, 0:w],
                bias_col[nm][:, jt:jt + 1])

        def proj_v_part(nm, dst, srcT, tt, h0, nh):
            # V-projection for a head subrange (nh <= 4 keeps the DoubleRow
            # moving free dim within 512); the 1/WS descale rides the
            # existing bias-add multiply
            wt_h = wsb_hi["v" if nm == "v" else "kv"]
            wt_l = wsb_lo["v" if nm == "v" else "kv"]
            ps = psproj.tile([P, 512], F32, name=f"psv{tt}_{h0}", tag="psj")
            mm = 0
            for wt, xs in ((wt_h, 0), (wt_h, 1), (wt_l, 0)):
                for hp in range(4):
                    nc.tensor.matmul(
                        ps[:, 0:nh * HD],
                        lhsT=srcT[xs][:, 2 * hp:2 * hp + 2,
                                      tt * P:(tt + 1) * P],
                        rhs=wt[:, 2 * hp:2 * hp + 2,
                               h0 * HD:(h0 + nh) * HD],
                        start=(mm == 0), stop=(mm == 11), perf_mode=DR)
                    mm += 1
            nc.vector.scalar_tensor_tensor(
                out=dst[:, tt, h0:h0 + nh, 0:HD],
                in0=ps[:, 0:nh * HD].rearrange("p (h d) -> p h d", h=nh),
                scalar=1.0 / WS,
                in1=bias_row[nm].rearrange(
                    "p (h d) -> p h d", h=NHL)[:, h0:h0 + nh, :],
                op0=ALU.mult, op1=ALU.add)
            nc.vector.tensor_copy(
                dst[:, tt, h0:h0 + nh, HD:HD + 1],
                twos.unsqueeze(1).broadcast_to([P, nh, 1]))

        # ---- prelude: Q/K jt0 in eight 256-wide groups, two phases of four
        # concurrent groups (two in the idle scores-psum slots); matmuls woven
        # in DMA-arrival order (set A needs only hi operands, set B needs
        # hsT_lo which lands last) so the PE starts and p-state-ramps early
        def prelude_phase(groups, single_start=False):
            """Q/K jt0 groups [(nm, sc, pool, tag)]: concurrent 256-wide
            accumulation groups, one PSUM bank each (psbig's banks are idle
            until the first scores matmul, so the first phase borrows them
            for two extra groups). Sets A (w_hi,x_hi) and C (w_lo,x_hi) are
            interleaved per hc-pair -- both consume the same arriving hsT_hi
            chunks, so interleaving doubles the ready work per DMA landing;
            set B (w_hi,x_lo) streams with the hsT_lo chunks. single_start
            emits set A's first hc pair as two plain single-hc fp8 matmuls
            so the PE starts as soon as (wqk0 hc0, hsT_hi hc0) land."""
            pre_ps = dict(groups)
            keys = [k for k, _ in groups]
            mm = {k: 0 for k in pre_ps}
            w = 256
            nmm = 13 if single_start else 12

            def emit(nm, sc, c0, xs, hp, single=None):
                k = (nm, sc)
                if single is None:
                    lhsT = wqk0_sb[:, 2 * hp:2 * hp + 2, c0:c0 + P]
                    rhs = hsT_sb[xs][:, 2 * hp:2 * hp + 2,
                                     sc * w:(sc + 1) * w]
                    pm = DR
                else:
                    lhsT = wqk0_sb[:, single, c0:c0 + P]
                    rhs = hsT_sb[xs][:, single, sc * w:(sc + 1) * w]
                    pm = None
                nc.tensor.matmul(pre_ps[k], lhsT=lhsT, rhs=rhs,
                                 start=(mm[k] == 0), stop=(mm[k] == nmm - 1),
                                 perf_mode=pm)
                mm[k] += 1

            hps_a = [0.5, 1, 2, 3] if single_start else [0, 1, 2, 3]
            for hp in hps_a:
                for nm, sc in keys:
                    ch, cl = qk_cols(nm, 0)[1:]
                    if hp == 0.5:
                        emit(nm, sc, ch, 0, 0, single=0)
                        emit(nm, sc, ch, 0, 0, single=1)
                        emit(nm, sc, cl, 0, 0)
                    else:
                        emit(nm, sc, ch, 0, hp)
                        if hp >= 1:
                            emit(nm, sc, cl, 0, hp)
            if not single_start:
                for nm, sc in keys:
                    ch, cl = qk_cols(nm, 0)[1:]
                    emit(nm, sc, cl, 0, 0)
            for hp in range(4):
                for nm, sc in keys:
                    ch, cl = qk_cols(nm, 0)[1:]
                    emit(nm, sc, ch, 1, hp)
            # bias-adds ordered so the first scores matmul's inputs (QT
            # s-half 0 and KT key-chunk 0) complete first
            for nm, sc in keys:
                dst = QT if nm == "q" else KT
                nc.vector.tensor_scalar_add(
                    dst[:, 0, sc * w:(sc + 1) * w],
                    pre_ps[(nm, sc)], bias_col[nm][:, 0:1])

        # all eight jt0 groups run in ONE phase: the two psbig tiles span two
        # banks each, so each hosts two groups in disjoint banks (cols 0-255
        # and 512-767); no group waits on another's bias-add
        pre_t1 = psproj.tile([P, 512], F32, name="pre_a", tag="psj")
        pre_t2 = psproj.tile([P, 512], F32, name="pre_b", tag="psj")
        pre_t3 = psctx.tile([P, 512], F32, name="pre_c", tag="ctxps")
        pre_t4 = psctx.tile([P, 512], F32, name="pre_d", tag="ctxps")
        pre_t5 = psbig.tile([P, S], F32, name="pre_e", tag="big")
        pre_t6 = psbig.tile([P, S], F32, name="pre_f", tag="big")
        prelude_phase([
            (("q", 0), pre_t1[:, 0:256]), (("k", 0), pre_t2[:, 0:256]),
            (("q", 1), pre_t3[:, 0:256]), (("k", 1), pre_t4[:, 0:256]),
            (("q", 2), pre_t5[:, 0:256]), (("k", 2), pre_t6[:, 0:256]),
            (("q", 3), pre_t5[:, 512:768]), (("k", 3), pre_t6[:, 512:768]),
        ], single_start=True)

        # ---- filler units: remaining projections, drained between branches
        def fill_unit(u):
            nm = u[0]
            if nm == "v":
                proj_v_part("v", Vaug, hsT_sb, u[1], u[2], u[3])
            elif nm == "kv":
                proj_v_part("kv", KVaug, ehsT_sb, u[1], u[2], u[3])
            elif nm == "kk":
                proj_t_unit("kk", KKT, ehsT_sb, u[1], u[2])
            elif nm == "q":
                proj_t_unit("q", QT, hsT_sb, u[1], u[2])
            elif nm == "k":
                proj_t_unit("k", KT, hsT_sb, u[1], u[2])
            elif nm == "kq":
                proj_t_unit("kq", KQT, hsT_sb, u[1], u[2])

        def sc_units(nm, jt, nsc):
            return [(nm, jt, sc) for sc in range(nsc)]

        # gap g -> units woven into the branch at schedule position g.
        # Units are woven INSIDE that branch's kt loop (paced across the kt
        # steps) so ready filler work sits between the ACT-dependent
        # scores/ctx matmuls in the PE queue; leftovers drain right after the
        # branch, which still meets every deadline. Schedule is
        # s0 s1 k0 k1 s2 s3 k2 k3 s4 s5 k4 k5 s6 s7 k6 k7; deadlines:
        # q/k jt before self(2jt); kq/kk jt before knl(2jt); V head h before
        # the branch after self(h) (its sflush); KV head h before the branch
        # after knl(h). Early gaps follow the DMA arrival order (wv, then
        # wkq/wkk/ehsT, then wkv, then wqkr).
        SCHED = "sk"
        # Unit order within each gap follows DMA arrival order (the weave is
        # a static PE instruction order, so a unit emitted before its weights
        # land head-of-line-blocks everything behind it).
        GAPS_SK = [
            # old (s,k)x6 + s6,s7,k6,k7 schedule; gap g consumed by branch at
            # position g
            [("v", t, 0, 4) for t in range(8)]
            + sc_units("kq", 0, 4) + sc_units("kk", 0, 2),   # G0 (s0)
            [("kv", t, 0, 4) for t in range(4)],             # G1 (k0)
            sc_units("q", 1, 4),                             # G2 (s1)
            sc_units("k", 1, 4),                             # G3 (k1)
            sc_units("kq", 1, 4) + sc_units("kk", 1, 2),     # G4 (s2)
            [("v", t, 4, 2) for t in range(8)],              # G5 (k2)
            sc_units("q", 2, 4),                             # G6 (s3)
            sc_units("k", 2, 4),                             # G7 (k3)
            sc_units("kq", 2, 4) + sc_units("kk", 2, 2),     # G8 (s4)
            [("kv", t, 4, 2) for t in range(4)],             # G9 (k4)
            sc_units("q", 3, 4),                             # G10 (s5)
            sc_units("k", 3, 4),                             # G11 (k5)
            sc_units("kq", 3, 4)
            + [("v", t, 6, 1) for t in range(8)],            # G12 (s6)
            [("kv", t, 6, 1) for t in range(4)],             # G13 (k6)
            sc_units("kk", 3, 2)
            + [("v", t, 7, 1) for t in range(8)],            # G14 (s7)
            [("kv", t, 7, 1) for t in range(4)],             # G15 (k7)
        ]
        GAPS = [
            # G0 (s0): V heads 0,1 (wv is the first weight to land), then
            # kq jt0 (wkq lands mid-s0; kq units sit at the late slots)
            [("v", t, 0, 1) for t in range(8)]
            + [("v", t, 1, 1) for t in range(8)]
            + sc_units("kq", 0, 4),
            # G1 (s1): V h2-3 first (wv resident), kk jt0 once ehsT+wkk have
            # landed mid-window
            [("v", t, 2, 2) for t in range(8)] + sc_units("kk", 0, 2),
            # G2 (k0): KV h0,h1 (wkv lands just before k0), then q jt1
            # (wqkr jt1 lands at k0's start)
            [("kv", t, 0, 1) for t in range(4)]
            + [("kv", t, 1, 1) for t in range(4)]
            + sc_units("q", 1, 4),
            sc_units("k", 1, 4),                           # G3 (k1): self2 needs jt1
            sc_units("kq", 1, 4) + sc_units("kk", 1, 2),   # G4 (s2): knl2 needs jt1
            [("v", t, 4, 2) for t in range(8)]
            + [("kv", t, 2, 2) for t in range(4)],         # G5 (s3)
            sc_units("q", 2, 4),                           # G6 (k2)
            sc_units("k", 2, 4),                           # G7 (k3): self4 needs jt2
            sc_units("kq", 2, 4) + sc_units("kk", 2, 2),   # G8 (s4): knl4 needs jt2
            [("kv", t, 4, 2) for t in range(4)]
            + sc_units("q", 3, 4),                         # G9 (s5)
            sc_units("k", 3, 4),                           # G10 (k4): self6 needs jt3
            sc_units("kq", 3, 2),                          # G11 (k5)
            [("v", t, 6, 1) for t in range(8)]
            + sc_units("kk", 3, 2)
            + [("v", t, 7, 1) for t in range(8)],          # G12 (s6)
            sc_units("kq", 3, 4)[2:]
            + [("kv", t, 6, 1) for t in range(4)]
            + [("kv", t, 7, 1) for t in range(4)],         # G13 (s7): knl6 needs jt3
            [],                                            # G14 (k6)
            [],                                            # G15 (k7)
        ]

        # ---- attention branches ----
        # `weave` is a list of (callable, pe_cycles) items (filler units,
        # previous-branch ctx flush parts, normalize/merge closures) emitted
        # across the kt steps paced by PE time: each exp step gets ~WEAVE_TGT
        # cycles of woven PE work, matching the ACT exp cadence so neither
        # engine runs dry.
        WEAVE_TGT = 1420.0  # (1038ns exp - 427ns scores) * 2.4 cycles/ns

        def head_branch(h, kt_mat, q_mat, vaug, n_keys, msk, weave,
                        split_last=False):
            base = (h % 2) * HD
            jt = h // 2
            nkt = n_keys // P
            ctxA = psctx.tile([P, 4, HD + 1], F32, name=f"cA_{h}_{n_keys}",
                              tag="ctxps")
            ctxB = psctx.tile([P, 4, HD + 1], F32, name=f"cB_{h}_{n_keys}",
                              tag="ctxps")

            nw = len(weave)
            slots = []
            cum = 0.0
            for fn, cost in weave:
                slots.append(min(nkt - 1, int(cum / WEAVE_TGT)))
                cum += cost
            e_ts = []
            for kt in range(nkt):
                st_ps = psbig.tile([P, S], F32, name=f"st_{h}_{kt}", tag="big")
                for sc2 in range(S // 512):
                    nc.tensor.matmul(
                        st_ps[:, sc2 * 512:(sc2 + 1) * 512],
                        lhsT=kt_mat[base:base + HD, jt, kt * P:(kt + 1) * P],
                        rhs=q_mat[base:base + HD, jt, sc2 * 512:(sc2 + 1) * 512],
                        start=True, stop=True)
                e_t = epool.tile([P, S], BF16, name=f"e_{h}_{kt}", tag="e")
                if split_last and kt >= nkt - 2:
                    # halve the last exp so the s<512 ctx groups (and the
                    # final normalize/merge/DMA chain) start half an exp early
                    for eh in range(2):
                        nc.scalar.activation(
                            e_t[:, eh * 512:(eh + 1) * 512],
                            st_ps[:, eh * 512:(eh + 1) * 512], AF.Exp,
                            bias=msk[:, kt:kt + 1], scale=INVS)
                else:
                    nc.scalar.activation(e_t, st_ps, AF.Exp,
                                         bias=msk[:, kt:kt + 1], scale=INVS)
                e_ts.append(e_t)
                for j in range(nw):
                    if slots[j] == kt:
                        weave[j][0]()

            def flush_part(scs):
                # sc-outer / kt-inner: one open accumulation group per PSUM
                # region at a time (interleaved groups corrupt each other)
                for sc in scs:
                    t = ctxA if sc < 4 else ctxB
                    for kt in range(nkt):
                        nc.tensor.matmul(
                            t[:, sc % 4, :],
                            lhsT=e_ts[kt][:, sc * P:(sc + 1) * P],
                            rhs=vaug[:, kt, h, :],
                            start=(kt == 0), stop=(kt == nkt - 1))

            flush_parts = [(lambda scs=(sc0, sc0 + 1): flush_part(scs),
                            2 * nkt * 65)
                           for sc0 in range(0, S // P, 2)]
            return (ctxA, ctxB), flush_parts

        def self_branch(h, weave):
            return head_branch(h, KT, QT, Vaug, TKS, mask_sb, weave)

        def knl_branch(h, weave, split_last=False):
            return head_branch(h, KKT, KQT, KVaug, TKK, emask_sb, weave,
                               split_last)

        def norm_part(h, t, i, dst):
            # dst[:, 4i:4i+4, :] = ctx-tile / (2*denominator)
            rb = smallp.tile([P, 4, 1], F32, name=f"rb_{h}_{i}", tag="rb",
                             bufs=4)
            nc.vector.reciprocal(rb, t[:, :, HD:HD + 1])
            nc.vector.tensor_tensor(
                out=dst[:, i * 4:i * 4 + 4, :], in0=t[:, :, 0:HD],
                in1=rb.broadcast_to([P, 4, HD]), op=ALU.mult)

        def out_dma(q, half):
            # head-pair quarter q: heads 2q, 2q+1 -> out columns [128q, 128q+128),
            # s-half `half` -> rows [512*half, 512*half+512)
            oh = out_half[q // 2]
            hp = (2 * q) % 4
            nc.sync.dma_start(
                out=out.ap()[half * 512:(half + 1) * 512,
                             q * P:(q + 1) * P].rearrange(
                    "(sc p) j -> p sc j", p=P),
                in_=oh[:, half * 4:(half + 1) * 4, hp:hp + 2, :].rearrange(
                    "p sc h d -> p sc (h d)"))

        # ---- main schedule: self(h) then knl(h); each branch weaves in the
        # previous branch's ctx flush + normalize/merge plus this gap's
        # projection units, so the ACT-bound exp chain is always overlapped
        # with ready PE work ----
        def unit_cost(u):
            if u[0] in ("v", "kv"):
                return 12 * u[3] * HD // 2
            return 12 * P
        def units(g):
            return [(lambda u=u: fill_unit(u), unit_cost(u)) for u in GAPS[g]]

        # self-self-knl-knl pairs: defers each knowledge branch's operand
        # needs (wkq/wkk/ehsT) a full window behind the DMA stream, and ends
        # on knl branches whose smaller exp-vs-PE deficit shrinks tail idle.
        # The "sk" variant is the baseline's (s,k) pairing with the same
        # knl-heavy tail.
        schedule = []
        if SCHED == "sskk":
            for hp2 in range(4):
                schedule += [("s", 2 * hp2), ("s", 2 * hp2 + 1),
                             ("k", 2 * hp2), ("k", 2 * hp2 + 1)]
        else:
            GAPS = GAPS_SK
            for h in range(6):
                schedule += [("s", h), ("k", h)]
            schedule += [("s", 6), ("s", 7), ("k", 6), ("k", 7)]
            # gap index by head as in the baseline: self h -> 2h, knl h ->
            # 2h+1 (so the tail order s6,s7,k6,k7 consumes G12,G14,G13,G15)
            gap_of = {("s", h): 2 * h for h in range(8)}
            gap_of.update({("k", h): 2 * h + 1 for h in range(8)})

        carry = []   # work woven into the next branch
        sN = {}
        for pos, (kind, h) in enumerate(schedule):
            g = pos if SCHED == "sskk" else gap_of[(kind, h)]
            if kind == "s":
                ctx_s, sfl = self_branch(h, units(g) + carry)

                # flush parts interleaved with the normalize halves they feed
                def mk_sn(i, h=h, ctx_s=ctx_s):
                    def f():
                        if i == 0:
                            sN[h] = snpool.tile([P, S // P, HD], F32,
                                                name=f"sN_{h}", tag="sN")
                        norm_part(h, ctx_s[i], i, sN[h])
                    return f

                carry = [sfl[0], sfl[1], (mk_sn(0), 0), sfl[2], sfl[3],
                         (mk_sn(1), 0)]
            else:
                ctx_k, kfl = knl_branch(h, units(g) + carry,
                                        split_last=(h == 7))

                tmp = {}

                def mk_k(i, h=h, ctx_k=ctx_k, tmp=tmp):
                    def f():
                        if i == 0:
                            tmp[0] = smallp.tile([P, S // P, HD], F32,
                                                 name=f"tK_{h}", tag="tK")
                        norm_part(h + 8, ctx_k[i], i, tmp[0])
                        oh = out_half[h // 4]
                        nc.vector.tensor_tensor(
                            out=oh[:, i * 4:i * 4 + 4, h % 4, :],
                            in0=tmp[0][:, i * 4:i * 4 + 4, :],
                            in1=sN[h][:, i * 4:i * 4 + 4, :], op=ALU.add)
                        if i == 1:
                            sN.pop(h)
                    return f

                carry = [kfl[0], kfl[1], (mk_k(0), 0), kfl[2], kfl[3],
                         (mk_k(1), 0)]
                if h % 2 == 1:
                    carry.insert(3, (lambda q=h // 2: out_dma(q, 0), 0))
                    carry.append((lambda q=h // 2: out_dma(q, 1), 0))
        for f, _ in carry:
            f()

    nc.finalize()
    return nc


def _get_nc():
    if "nc" not in _CACHE:
        _CACHE["nc"] = _build()
    return _CACHE["nc"]


def kernel(**inputs):
    inp = {k: np.asarray(v, dtype=np.float32) for k, v in inputs.items()}
    nc = _get_nc()

    E4 = ml_dtypes.float8_e4m3

    def f8(x):
        return np.ascontiguousarray(np.asarray(x).astype(E4))

    def hilo(x):
        hi = np.asarray(x).astype(E4)
        lo = (np.asarray(x) - hi.astype(np.float32)).astype(E4)
        return np.ascontiguousarray(hi), np.ascontiguousarray(lo)

    B = 4
    in_maps = []
    for core in range(8):
        b, hg = core // 2, core % 2
        sl = slice(hg * HG, (hg + 1) * HG)
        hsT = inp["hidden_states"][b].T
        ehsT = inp["encoder_hidden_states"][b].T
        hs_hi, hs_lo = hilo(hsT)
        ehs_hi, ehs_lo = hilo(ehsT)
        wqh, wql = hilo(inp["Wq"][:, sl] * WS)
        wkh, wkl = hilo(inp["Wk"][:, sl] * WS)
        m = {
            "hsT_hi": hs_hi, "hsT_lo": hs_lo,
            "ehsT_hi": ehs_hi, "ehsT_lo": ehs_lo,
            "wqk0": np.ascontiguousarray(np.concatenate(
                [wqh[:, 0:P], wkh[:, 0:P], wql[:, 0:P], wkl[:, 0:P]],
                axis=1)),
            "wqkr": np.ascontiguousarray(np.concatenate(
                [blk[:, jt * P:(jt + 1) * P]
                 for jt in range(1, 4)
                 for blk in (wqh, wkh, wql, wkl)], axis=1)),
        }
        for nm in ["v", "kq", "kk", "kv"]:
            hi, lo = hilo(inp[f"W{nm}"][:, sl] * WS)
            m[f"w{nm}_hi"] = hi
            m[f"w{nm}_lo"] = lo
        # packed small constants: QT/KT stay x32-scaled until the exp, so
        # the q/k/kq/kk biases scale to match
        def pjt(x, n):
            return np.asarray(x).reshape(n, P).T
        c0 = np.concatenate([
            pjt(inp["bq"][sl] * WS, 4), pjt(inp["bk"][sl] * WS, 4),
            pjt(inp["attention_mask"][b, 0, 0, :], 8),
            pjt(inp["encoder_attention_mask"][b, 0, 0, :], 4),
            pjt(inp["bkq"][sl] * WS, 4), pjt(inp["bkk"][sl] * WS, 4),
        ], axis=1).astype(np.float32)
        c1 = np.concatenate([
            np.broadcast_to(inp["bv"][sl], (P, HG)),
            np.broadcast_to(inp["bkv"][sl], (P, HG)),
        ], axis=1).astype(np.float32)
        m["c0"] = np.ascontiguousarray(c0)
        m["c1"] = np.ascontiguousarray(c1)
        in_maps.append(m)

    res = run_bass_kernel_spmd(nc, in_maps, core_ids=list(range(8)))

    outp = np.empty((B, S, H), np.float32)
    for core in range(8):
        b, hg = core // 2, core % 2
        outp[b, :, hg * HG:(hg + 1) * HG] = res.results[core]["out"]
    return outp


# revision 80
# speedup vs baseline: 1.0014x; 1.0014x over previous
"""Trainium2 Bass kernel for nn_BertSelfAttention_7962869367489.

Dual-branch (self + cross/"knowledge") BERT attention, B=4, S=1024, K=512,
H=1024, NH=16, HD=64, fp32.

Sharding: 8 cores = (batch b in 0..3) x (head-group hg in 0..1, 8 heads each).
All six projections are column-split by head-group; per-head attention is
entirely core-local; output columns are disjoint per core, so the gather is a
pure concatenation (no collectives).

Per-core pipeline:
  - All six projections run as fp8e4m3 DoubleRow matmuls (0.5 PE rows per
    output element, 2 contraction chunks per instruction) using a hi/lo
    residual split prepared on the host: X_hi=fp8(X), X_lo=fp8(X-X_hi) (raw,
    using fp8 subnormals), W'=32*W, W_hi=fp8(W'), W_lo=fp8(W'-W_hi). The
    three product sets hi*hi + lo_x*hi + hi*lo_w share one product scale (32)
    and accumulate in a single PSUM group; measured accuracy matches bf16.
    The x32 descale folds into the exp scale (QK side, /1024) and into the
    V bias-add multiply (1/32), so no extra DVE work.
  - Projections: QT/KT/KQT/KKT = W.T @ srcT in transposed orientation (bf16
    outs, carrying the x32 scale); Vaug/KVaug = srcT.T @ Wv in normal
    orientation with an augmented column of 2.0, so the ctx matmul also
    produces 2*softmax-denominator, folding the (ctx+kctx)*0.5 branch average
    into the normalization.
  - Per head h: scoresT[t,s] = K_h @ Q_h^T (bf16, contraction HD=64); exp on
    ACT with per-partition mask bias and 1/(8*1024) scale, written bf16;
    after the branch's exps, ctx[s,d|den] accumulates in PSUM in NORMAL
    orientation via lhsT = e-chunk [t,128s], rhs = Vaug_h [t,65] (bf16).
    Accumulation groups run sc-outer/kt-inner so each PSUM region hosts one
    group at a time (interleaved groups in one bank clobber each other).
  - Normalization + branch merge on DVE straight out of PSUM; output DMA'd
    in four head-pair quarters.
  - Remaining projections are split into ~0.64us (jt, sc) units and woven
    between attention branches, paced by PE time (~1420 cycles of filler
    per exp step) so the PE never outruns the ACT exp cadence; knowledge
    branch h runs right after self branch h, with the last four branches
    ordered s6 s7 k6 k7 (knl branches have the smaller exp deficit).
  - Unit order inside each weave gap follows the DMA arrival order: the
    weave is a static PE instruction order, so a unit emitted before its
    weights land head-of-line-blocks everything behind it. DMA chunks are
    ~0.25-0.5MB (smaller is HWDGE-issue-bound at ~650ns/DMA); all small
    constants ship as one host-packed array.
"""
import numpy as np
import ml_dtypes
from contextlib import ExitStack

import concourse.bacc as bacc
import concourse.tile as tile
import concourse.mybir as mybir
from concourse.bass_utils import run_bass_kernel_spmd

F32 = mybir.dt.float32
BF16 = mybir.dt.bfloat16
F8 = mybir.dt.float8e4
AF = mybir.ActivationFunctionType
ALU = mybir.AluOpType
DR = mybir.MatmulPerfMode.DoubleRow

P = 128
S = 1024        # query length
TKS = 1024      # self-branch key length
TKK = 512       # knowledge-branch key length
H = 1024        # model dim (projection contraction)
HG = 512        # per-core output width (8 heads x 64)
NHL = 8         # heads per core
HD = 64
HC = H // P     # 8 contraction chunks
WS = 32.0       # host-side weight prescale (fp8 range centering)
INV = 0.125     # 1/sqrt(64)
INVS = INV / (WS * WS)  # exp scale absorbing the x32 on both Q and K

_CACHE = {}


def _build():
    nc = bacc.Bacc(target_bir_lowering=False, debug=False)

    hsT_hi = nc.dram_tensor("hsT_hi", [H, S], F8, kind="ExternalInput")
    hsT_lo = nc.dram_tensor("hsT_lo", [H, S], F8, kind="ExternalInput")
    ehsT_hi = nc.dram_tensor("ehsT_hi", [H, TKK], F8, kind="ExternalInput")
    ehsT_lo = nc.dram_tensor("ehsT_lo", [H, TKK], F8, kind="ExternalInput")
    # host-packed [wq_hi jt0 | wk_hi jt0 | wq_lo jt0 | wk_lo jt0]: contiguous
    # 512B rows dodge the sub-512B-run DMA penalty on the startup-critical
    # first weight loads
    wqk0 = nc.dram_tensor("wqk0", [H, 4 * P], F8, kind="ExternalInput")
    # host-packed [wq_hi jt1-3 | wk_hi jt1-3 | wq_lo jt1-3 | wk_lo jt1-3]:
    # 1536B rows, one clean DMA for the rest of the q/k weights
    wqkr = nc.dram_tensor("wqkr", [H, 4 * 3 * P], F8, kind="ExternalInput")
    w_hi = {}
    w_lo = {}
    b_in = {}
    for nm in ["v", "kq", "kk", "kv"]:
        w_hi[nm] = nc.dram_tensor(f"w{nm}_hi", [H, HG], F8,
                                  kind="ExternalInput")
        w_lo[nm] = nc.dram_tensor(f"w{nm}_lo", [H, HG], F8,
                                  kind="ExternalInput")
    # host-packed small constants, one DMA slot: [bias_q(4) | bias_k(4) |
    # mask(8) | emask(4) | bias_kq(4) | bias_kk(4)] in (p, col) layout
    c0_in = nc.dram_tensor("c0", [P, 28], F32, kind="ExternalInput")
    # host-packed broadcast rows: [bias_v(512) | bias_kv(512)]
    c1_in = nc.dram_tensor("c1", [P, 2 * HG], F32, kind="ExternalInput")
    out = nc.dram_tensor("out", [S, HG], F32, kind="ExternalOutput")

    with tile.TileContext(nc) as tc, ExitStack() as ctx:
        const = ctx.enter_context(tc.tile_pool(name="const", bufs=1))
        persist = ctx.enter_context(tc.tile_pool(name="persist", bufs=1))
        epool = ctx.enter_context(tc.tile_pool(name="epool", bufs=17))
        smallp = ctx.enter_context(tc.tile_pool(name="smallp", bufs=2))
        snpool = ctx.enter_context(tc.tile_pool(name="snpool", bufs=3))
        psproj = ctx.enter_context(tc.tile_pool(name="psproj", bufs=2, space="PSUM"))
        psbig = ctx.enter_context(tc.tile_pool(name="psbig", bufs=2, space="PSUM"))
        psctx = ctx.enter_context(tc.tile_pool(name="psctx", bufs=2, space="PSUM"))

        # ---- constants (views into the two host-packed tiles) ----
        c0_sb = const.tile([P, 28], F32)
        c1_sb = const.tile([P, 2 * HG], F32)
        bias_col = {"q": c0_sb[:, 0:4], "k": c0_sb[:, 4:8],
                    "kq": c0_sb[:, 20:24], "kk": c0_sb[:, 24:28]}
        mask_sb = c0_sb[:, 8:16]
        emask_sb = c0_sb[:, 16:20]
        bias_row = {"v": c1_sb[:, 0:HG], "kv": c1_sb[:, HG:2 * HG]}
        twos = const.tile([P, 1], F32)
        nc.vector.memset(twos, 2.0)

        def load_consts_early():
            nc.sync.dma_start(out=c0_sb, in_=c0_in.ap())

        def load_consts_late():
            nc.sync.dma_start(out=c1_sb, in_=c1_in.ap())

        # ---- persistent activations ----
        QT = persist.tile([P, 4, S], BF16)       # [j%128, jt, s] (x32 scaled)
        KT = persist.tile([P, 4, TKS], BF16)
        KQT = persist.tile([P, 4, S], BF16)
        KKT = persist.tile([P, 4, TKK], BF16)
        Vaug = persist.tile([P, TKS // P, NHL, HD + 1], BF16)   # [t%128, tt, h, d|2]
        KVaug = persist.tile([P, TKK // P, NHL, HD + 1], BF16)
        hsT_sb = {0: persist.tile([P, HC, S], F8, name="hsT_hi_sb"),
                  1: persist.tile([P, HC, S], F8, name="hsT_lo_sb")}
        ehsT_sb = {0: persist.tile([P, HC, TKK], F8, name="ehsT_hi_sb"),
                   1: persist.tile([P, HC, TKK], F8, name="ehsT_lo_sb")}
        wsb_hi = {}
        wsb_lo = {}
        for nm in ["v", "kq", "kk", "kv"]:
            wsb_hi[nm] = persist.tile([P, HC, HG], F8, name=f"wh_{nm}")
            wsb_lo[nm] = persist.tile([P, HC, HG], F8, name=f"wl_{nm}")
        wqk0_sb = persist.tile([P, HC, 4 * P], F8)
        wqkr_sb = persist.tile([P, HC, 4 * 3 * P], F8)
        # output staging in two head-halves; DMA'd in four head-pair quarters
        out_half = [persist.tile([P, S // P, 4, HD], F32, name=f"out_half{i}",
                                 tag=f"out_half{i}") for i in range(2)]

        # q/k weight operand lookup: (tile, hi col base, lo col base) per jt.
        # wqkr is packed per-jt: [jt block][qh|kh|ql|kl] so each jt's weights
        # load as one contiguous-row DMA in deadline order.
        def qk_cols(nm, jt):
            off = 0 if nm == "q" else 1
            if jt == 0:
                return wqk0_sb, off * P, (2 + off) * P
            base = (jt - 1) * 4 * P
            return wqkr_sb, base + off * P, base + (2 + off) * P

        # ---- input DMAs (sync/HWDGE queue), ordered so the prelude's
        # dependencies (wqk0, hsT_hi, then hsT_lo) land first ----
        def load_rows(dst, src, half, rows, cols):
            nc.sync.dma_start(
                out=dst[:, half * (rows // 2):(half + 1) * (rows // 2), :],
                in_=src[half * (rows * P // 2):(half + 1) * (rows * P // 2), :]
                .rearrange("(hc p) s -> p hc s", p=P))

        def load_w(nm, which):
            src = w_hi[nm] if which == 0 else w_lo[nm]
            dst = wsb_hi[nm] if which == 0 else wsb_lo[nm]
            nc.sync.dma_start(
                out=dst, in_=src.ap().rearrange("(hc p) j -> p hc j", p=P))

        def load_wqk0(hc0, hcn):
            nc.sync.dma_start(
                out=wqk0_sb[:, hc0:hc0 + hcn, :],
                in_=wqk0[hc0 * P:(hc0 + hcn) * P, :].rearrange(
                    "(hc p) j -> p hc j", p=P))

        def load_hsT(which, hc0, hcn):
            src = hsT_hi if which == 0 else hsT_lo
            nc.sync.dma_start(
                out=hsT_sb[which][:, hc0:hc0 + hcn, :],
                in_=src[hc0 * P:(hc0 + hcn) * P, :].rearrange(
                    "(hc p) s -> p hc s", p=P))

        # startup-critical loads in strict first-use order: the prelude's
        # set-A matmuls need wqk0(hi cols) + hsT_hi (first two chunks feed
        # single-hc fp8 matmuls so the PE starts before the pair lands);
        # set-C needs wqk0(lo cols); set-B needs hsT_lo. Then weights in
        # weave-consumption order: wv (G0's V units), wkq/wkk+ehsT (knl0),
        # wkv (kv units), wqkr (q/k jt1-3).
        # 2-hc (0.25MB) chunks: smaller chunks are HWDGE-issue-bound (625ns
        # hold per DMA), larger ones delay the first matmul
        load_wqk0(0, 2)
        load_hsT(0, 0, 2)
        for hc2 in (2, 4, 6):
            load_wqk0(hc2, 2)
            load_hsT(0, hc2, 2)
        load_consts_early()
        # hsT_lo in fine chunks: the prelude's set-B hc-pairs stream as they
        # land instead of waiting for one big transfer
        load_hsT(1, 0, 2)
        load_hsT(1, 2, 2)
        load_hsT(1, 4, 2)
        load_hsT(1, 6, 2)
        load_w("v", 0)
        load_w("v", 1)
        load_consts_late()
        load_w("kq", 0)
        load_w("kq", 1)
        load_rows(ehsT_sb[0], ehsT_hi, 0, HC, TKK)
        load_rows(ehsT_sb[0], ehsT_hi, 1, HC, TKK)
        load_w("kk", 0)
        load_w("kk", 1)
        load_rows(ehsT_sb[1], ehsT_lo, 0, HC, TKK)
        load_rows(ehsT_sb[1], ehsT_lo, 1, HC, TKK)
        load_w("kv", 0)
        load_w("kv", 1)
        for jt in range(3):
            nc.sync.dma_start(
                out=wqkr_sb[:, :, jt * 4 * P:(jt + 1) * 4 * P],
                in_=wqkr[:, jt * 4 * P:(jt + 1) * 4 * P].rearrange(
                    "(hc p) j -> p hc j", p=P))

        # ---- projection emitters (fp8 hi/lo DoubleRow, single PSUM group:
        # sets (w_hi,x_hi), (w_hi,x_lo), (w_lo,x_hi), 4 hc-pairs each) ----
        def proj_t_unit(nm, dst, srcT, jt, sc, w=256):
            if nm in ("q", "k"):
                wt_h, ch, cl = qk_cols(nm, jt)
                wt_l = wt_h
            else:
                wt_h, wt_l = wsb_hi[nm], wsb_lo[nm]
                ch = cl = jt * P
            ps = psproj.tile([P, 512], F32, name="psj", tag="psj")
            mm = 0
            for wt, c0, xs in ((wt_h, ch, 0), (wt_h, ch, 1), (wt_l, cl, 0)):
                for hp in range(4):
                    nc.tensor.matmul(
                        ps[:, 0:w],
                        lhsT=wt[:, 2 * hp:2 * hp + 2, c0:c0 + P],
                        rhs=srcT[xs][:, 2 * hp:2 * hp + 2,
                                     sc * w:(sc + 1) * w],
                        start=(mm == 0), stop=(mm == 11), perf_mode=DR)
                    mm += 1
            nc.vector.tensor_scalar_add(
                dst[:, jt, sc * w:(sc + 1) * w], ps[:, 0:w],
                bias_col[nm][:, jt:jt + 1])

        def proj_v_part(nm, dst, srcT, tt, h0, nh):
            # V-projection for a head subrange (nh <= 4 keeps the DoubleRow
            # moving free dim within 512); the 1/WS descale rides the
            # existing bias-add multiply
            wt_h = wsb_hi["v" if nm == "v" else "kv"]
            wt_l = wsb_lo["v" if nm == "v" else "kv"]
            ps = psproj.tile([P, 512], F32, name=f"psv{tt}_{h0}", tag="psj")
            mm = 0
            for wt, xs in ((wt_h, 0), (wt_h, 1), (wt_l, 0)):
                for hp in range(4):
                    nc.tensor.matmul(
                        ps[:, 0:nh * HD],
                        lhsT=srcT[xs][:, 2 * hp:2 * hp + 2,
                                      tt * P:(tt + 1) * P],
                        rhs=wt[:, 2 * hp:2 * hp + 2,
                               h0 * HD:(h0 + nh) * HD],
                        start=(mm == 0), stop=(mm == 11), perf_mode=DR)
                    mm += 1
            nc.vector.scalar_tensor_tensor(
                out=dst[:, tt, h0:h0 + nh, 0:HD],
                in0=ps[:, 0:nh * HD].rearrange("p (h d) -> p h d", h=nh),
                scalar=1.0 / WS,
                in1=bias_row[nm].rearrange(
                    "p (h d) -> p h d", h=NHL)[:, h0:h0 + nh, :],
                op0=ALU.mult, op1=ALU.add)
            nc.vector.tensor_copy(
                dst[:, tt, h0:h0 + nh, HD:HD + 1],
                twos.unsqueeze(1).broadcast_to([P, nh, 1]))

        # ---- prelude: Q/K jt0 in eight 256-wide groups, two phases of four
        # concurrent groups (two in the idle scores-psum slots); matmuls woven
        # in DMA-arrival order (set A needs only hi operands, set B needs
        # hsT_lo which lands last) so the PE starts and p-state-ramps early
        def prelude_phase(groups, single_start=False):
            """Q/K jt0 groups [(nm, sc, pool, tag)]: concurrent 256-wide
            accumulation groups, one PSUM bank each (psbig's banks are idle
            until the first scores matmul, so the first phase borrows them
            for two extra groups). Sets A (w_hi,x_hi) and C (w_lo,x_hi) are
            interleaved per hc-pair -- both consume the same arriving hsT_hi
            chunks, so interleaving doubles the ready work per DMA landing;
            set B (w_hi,x_lo) streams with the hsT_lo chunks. single_start
            emits set A's first hc pair as two plain single-hc fp8 matmuls
            so the PE starts as soon as (wqk0 hc0, hsT_hi hc0) land."""
            pre_ps = dict(groups)
            keys = [k for k, _ in groups]
            mm = {k: 0 for k in pre_ps}
            w = 256
            nmm = 13 if single_start else 12

            def emit(nm, sc, c0, xs, hp, single=None):
                k = (nm, sc)
                if single is None:
                    lhsT = wqk0_sb[:, 2 * hp:2 * hp + 2, c0:c0 + P]
                    rhs = hsT_sb[xs][:, 2 * hp:2 * hp + 2,
                                     sc * w:(sc + 1) * w]
                    pm = DR
                else:
                    lhsT = wqk0_sb[:, single, c0:c0 + P]
                    rhs = hsT_sb[xs][:, single, sc * w:(sc + 1) * w]
                    pm = None
                nc.tensor.matmul(pre_ps[k], lhsT=lhsT, rhs=rhs,
                                 start=(mm[k] == 0), stop=(mm[k] == nmm - 1),
                                 perf_mode=pm)
                mm[k] += 1

            hps_a = [0.5, 1, 2, 3] if single_start else [0, 1, 2, 3]
            for hp in hps_a:
                for nm, sc in keys:
                    ch, cl = qk_cols(nm, 0)[1:]
                    if hp == 0.5:
                        emit(nm, sc, ch, 0, 0, single=0)
                        emit(nm, sc, ch, 0, 0, single=1)
                        emit(nm, sc, cl, 0, 0)
                    else:
                        emit(nm, sc, ch, 0, hp)
                        if hp >= 1:
                            emit(nm, sc, cl, 0, hp)
            if not single_start:
                for nm, sc in keys:
                    ch, cl = qk_cols(nm, 0)[1:]
                    emit(nm, sc, cl, 0, 0)
            for hp in range(4):
                for nm, sc in keys:
                    ch, cl = qk_cols(nm, 0)[1:]
                    emit(nm, sc, ch, 1, hp)
            # bias-adds ordered so the first scores matmul's inputs (QT
            # s-half 0 and KT key-chunk 0) complete first
            for nm, sc in keys:
                dst = QT if nm == "q" else KT
                nc.vector.tensor_scalar_add(
                    dst[:, 0, sc * w:(sc + 1) * w],
                    pre_ps[(nm, sc)], bias_col[nm][:, 0:1])

        # all eight jt0 groups run in ONE phase: the two psbig tiles span two
        # banks each, so each hosts two groups in disjoint banks (cols 0-255
        # and 512-767); no group waits on another's bias-add
        pre_t1 = psproj.tile([P, 512], F32, name="pre_a", tag="psj")
        pre_t2 = psproj.tile([P, 512], F32, name="pre_b", tag="psj")
        pre_t3 = psctx.tile([P, 512], F32, name="pre_c", tag="ctxps")
        pre_t4 = psctx.tile([P, 512], F32, name="pre_d", tag="ctxps")
        pre_t5 = psbig.tile([P, S], F32, name="pre_e", tag="big")
        pre_t6 = psbig.tile([P, S], F32, name="pre_f", tag="big")
        prelude_phase([
            (("q", 0), pre_t1[:, 0:256]), (("k", 0), pre_t2[:, 0:256]),
            (("q", 1), pre_t3[:, 0:256]), (("k", 1), pre_t4[:, 0:256]),
            (("q", 2), pre_t5[:, 0:256]), (("k", 2), pre_t6[:, 0:256]),
            (("q", 3), pre_t5[:, 512:768]), (("k", 3), pre_t6[:, 512:768]),
        ], single_start=True)

        # ---- filler units: remaining projections, drained between branches
        def fill_unit(u):
            nm = u[0]
            if nm == "v":
                proj_v_part("v", Vaug, hsT_sb, u[1], u[2], u[3])
            elif nm == "kv":
                proj_v_part("kv", KVaug, ehsT_sb, u[1], u[2], u[3])
            elif nm == "kk":
                proj_t_unit("kk", KKT, ehsT_sb, u[1], u[2])
            elif nm == "q":
                proj_t_unit("q", QT, hsT_sb, u[1], u[2])
            elif nm == "k":
                proj_t_unit("k", KT, hsT_sb, u[1], u[2])
            elif nm == "kq":
                proj_t_unit("kq", KQT, hsT_sb, u[1], u[2])

        def sc_units(nm, jt, nsc):
            return [(nm, jt, sc) for sc in range(nsc)]

        # gap g -> units woven into the branch at schedule position g.
        # Units are woven INSIDE that branch's kt loop (paced across the kt
        # steps) so ready filler work sits between the ACT-dependent
        # scores/ctx matmuls in the PE queue; leftovers drain right after the
        # branch, which still meets every deadline. Schedule is
        # s0 s1 k0 k1 s2 s3 k2 k3 s4 s5 k4 k5 s6 s7 k6 k7; deadlines:
        # q/k jt before self(2jt); kq/kk jt before knl(2jt); V head h before
        # the branch after self(h) (its sflush); KV head h before the branch
        # after knl(h). Early gaps follow the DMA arrival order (wv, then
        # wkq/wkk/ehsT, then wkv, then wqkr).
        SCHED = "sk"
        # Unit order within each gap follows DMA arrival order (the weave is
        # a static PE instruction order, so a unit emitted before its weights
        # land head-of-line-blocks everything behind it).
        GAPS_SK = [
            # old (s,k)x6 + s6,s7,k6,k7 schedule; gap g consumed by branch at
            # position g
            [("v", t, 0, 4) for t in range(8)]
            + sc_units("kq", 0, 4) + sc_units("kk", 0, 2),   # G0 (s0)
            [("v", t, 4, 2) for t in range(8)],              # G1 (k0)
            [("kv", t, 0, 4) for t in range(4)]
            + sc_units("q", 1, 4),                           # G2 (s1)
            sc_units("k", 1, 4),                             # G3 (k1)
            sc_units("kq", 1, 4) + sc_units("kk", 1, 2),     # G4 (s2)
            [],                                              # G5 (k2)
            sc_units("q", 2, 4),                             # G6 (s3)
            sc_units("k", 2, 4),                             # G7 (k3)
            sc_units("kq", 2, 4) + sc_units("kk", 2, 2),     # G8 (s4)
            [("kv", t, 4, 2) for t in range(4)],             # G9 (k4)
            sc_units("q", 3, 4),                             # G10 (s5)
            sc_units("k", 3, 4),                             # G11 (k5)
            sc_units("kq", 3, 4)
            + [("v", t, 6, 1) for t in range(8)],            # G12 (s6)
            [("kv", t, 6, 1) for t in range(4)],             # G13 (k6)
            sc_units("kk", 3, 2)
            + [("v", t, 7, 1) for t in range(8)],            # G14 (s7)
            [("kv", t, 7, 1) for t in range(4)],             # G15 (k7)
        ]
        GAPS = [
            # G0 (s0): V heads 0,1 (wv is the first weight to land), then
            # kq jt0 (wkq lands mid-s0; kq units sit at the late slots)
            [("v", t, 0, 1) for t in range(8)]
            + [("v", t, 1, 1) for t in range(8)]
            + sc_units("kq", 0, 4),
            # G1 (s1): V h2-3 first (wv resident), kk jt0 once ehsT+wkk have
            # landed mid-window
            [("v", t, 2, 2) for t in range(8)] + sc_units("kk", 0, 2),
            # G2 (k0): KV h0,h1 (wkv lands just before k0), then q jt1
            # (wqkr jt1 lands at k0's start)
            [("kv", t, 0, 1) for t in range(4)]
            + [("kv", t, 1, 1) for t in range(4)]
            + sc_units("q", 1, 4),
            sc_units("k", 1, 4),                           # G3 (k1): self2 needs jt1
            sc_units("kq", 1, 4) + sc_units("kk", 1, 2),   # G4 (s2): knl2 needs jt1
            [("v", t, 4, 2) for t in range(8)]
            + [("kv", t, 2, 2) for t in range(4)],         # G5 (s3)
            sc_units("q", 2, 4),                           # G6 (k2)
            sc_units("k", 2, 4),                           # G7 (k3): self4 needs jt2
            sc_units("kq", 2, 4) + sc_units("kk", 2, 2),   # G8 (s4): knl4 needs jt2
            [("kv", t, 4, 2) for t in range(4)]
            + sc_units("q", 3, 4),                         # G9 (s5)
            sc_units("k", 3, 4),                           # G10 (k4): self6 needs jt3
            sc_units("kq", 3, 2),                          # G11 (k5)
            [("v", t, 6, 1) for t in range(8)]
            + sc_units("kk", 3, 2)
            + [("v", t, 7, 1) for t in range(8)],          # G12 (s6)
            sc_units("kq", 3, 4)[2:]
            + [("kv", t, 6, 1) for t in range(4)]
            + [("kv", t, 7, 1) for t in range(4)],         # G13 (s7): knl6 needs jt3
            [],                                            # G14 (k6)
            [],                                            # G15 (k7)
        ]

        # ---- attention branches ----
        # `weave` is a list of (callable, pe_cycles) items (filler units,
        # previous-branch ctx flush parts, normalize/merge closures) emitted
        # across the kt steps paced by PE time: each exp step gets ~WEAVE_TGT
        # cycles of woven PE work, matching the ACT exp cadence so neither
        # engine runs dry.
        WEAVE_TGT = 1420.0  # (1038ns exp - 427ns scores) * 2.4 cycles/ns

        def head_branch(h, kt_mat, q_mat, vaug, n_keys, msk, weave,
                        split_last=False):
            base = (h % 2) * HD
            jt = h // 2
            nkt = n_keys // P
            ctxA = psctx.tile([P, 4, HD + 1], F32, name=f"cA_{h}_{n_keys}",
                              tag="ctxps")
            ctxB = psctx.tile([P, 4, HD + 1], F32, name=f"cB_{h}_{n_keys}",
                              tag="ctxps")

            nw = len(weave)
            slots = []
            cum = 0.0
            for fn, cost in weave:
                slots.append(min(nkt - 1, int(cum / WEAVE_TGT)))
                cum += cost
            e_ts = []
            for kt in range(nkt):
                st_ps = psbig.tile([P, S], F32, name=f"st_{h}_{kt}", tag="big")
                for sc2 in range(S // 512):
                    nc.tensor.matmul(
                        st_ps[:, sc2 * 512:(sc2 + 1) * 512],
                        lhsT=kt_mat[base:base + HD, jt, kt * P:(kt + 1) * P],
                        rhs=q_mat[base:base + HD, jt, sc2 * 512:(sc2 + 1) * 512],
                        start=True, stop=True)
                e_t = epool.tile([P, S], BF16, name=f"e_{h}_{kt}", tag="e")
                if split_last and kt >= nkt - 2:
                    # halve the last exp so the s<512 ctx groups (and the
                    # final normalize/merge/DMA chain) start half an exp early
                    for eh in range(2):
                        nc.scalar.activation(
                            e_t[:, eh * 512:(eh + 1) * 512],
                            st_ps[:, eh * 512:(eh + 1) * 512], AF.Exp,
                            bias=msk[:, kt:kt + 1], scale=INVS)
                else:
                    nc.scalar.activation(e_t, st_ps, AF.Exp,
                                         bias=msk[:, kt:kt + 1], scale=INVS)
                e_ts.append(e_t)
                for j in range(nw):
                    if slots[j] == kt:
                        weave[j][0]()

            def flush_part(scs):
                # sc-outer / kt-inner: one open accumulation group per PSUM
                # region at a time (interleaved groups corrupt each other)
                for sc in scs:
                    t = ctxA if sc < 4 else ctxB
                    for kt in range(nkt):
                        nc.tensor.matmul(
                            t[:, sc % 4, :],
                            lhsT=e_ts[kt][:, sc * P:(sc + 1) * P],
                            rhs=vaug[:, kt, h, :],
                            start=(kt == 0), stop=(kt == nkt - 1))

            flush_parts = [(lambda scs=(sc0, sc0 + 1): flush_part(scs),
                            2 * nkt * 65)
                           for sc0 in range(0, S // P, 2)]
            return (ctxA, ctxB), flush_parts

        def self_branch(h, weave):
            return head_branch(h, KT, QT, Vaug, TKS, mask_sb, weave)

        def knl_branch(h, weave, split_last=False):
            return head_branch(h, KKT, KQT, KVaug, TKK, emask_sb, weave,
                               split_last)

        def norm_part(h, t, i, dst):
            # dst[:, 4i:4i+4, :] = ctx-tile / (2*denominator)
            rb = smallp.tile([P, 4, 1], F32, name=f"rb_{h}_{i}", tag="rb",
                             bufs=4)
            nc.vector.reciprocal(rb, t[:, :, HD:HD + 1])
            nc.vector.tensor_tensor(
                out=dst[:, i * 4:i * 4 + 4, :], in0=t[:, :, 0:HD],
                in1=rb.broadcast_to([P, 4, HD]), op=ALU.mult)

        def out_dma(q, half):
            # head-pair quarter q: heads 2q, 2q+1 -> out columns [128q, 128q+128),
            # s-half `half` -> rows [512*half, 512*half+512)
            oh = out_half[q // 2]
            hp = (2 * q) % 4
            nc.sync.dma_start(
                out=out.ap()[half * 512:(half + 1) * 512,
                             q * P:(q + 1) * P].rearrange(
                    "(sc p) j -> p sc j", p=P),
                in_=oh[:, half * 4:(half + 1) * 4, hp:hp + 2, :].rearrange(
                    "p sc h d -> p sc (h d)"))

        # ---- main schedule: self(h) then knl(h); each branch weaves in the
        # previous branch's ctx flush + normalize/merge plus this gap's
        # projection units, so the ACT-bound exp chain is always overlapped
        # with ready PE work ----
        def unit_cost(u):
            if u[0] in ("v", "kv"):
                return 12 * u[3] * HD // 2
            return 12 * P
        def units(g):
            return [(lambda u=u: fill_unit(u), unit_cost(u)) for u in GAPS[g]]

        # self-self-knl-knl pairs: defers each knowledge branch's operand
        # needs (wkq/wkk/ehsT) a full window behind the DMA stream, and ends
        # on knl branches whose smaller exp-vs-PE deficit shrinks tail idle.
        # The "sk" variant is the baseline's (s,k) pairing with the same
        # knl-heavy tail.
        schedule = []
        if SCHED == "sskk":
            for hp2 in range(4):
                schedule += [("s", 2 * hp2), ("s", 2 * hp2 + 1),
                             ("k", 2 * hp2), ("k", 2 * hp2 + 1)]
        else:
            GAPS = GAPS_SK
            for h in range(6):
                schedule += [("s", h), ("k", h)]
            schedule += [("s", 6), ("s", 7), ("k", 6), ("k", 7)]
            # gap index by head as in the baseline: self h -> 2h, knl h ->
            # 2h+1 (so the tail order s6,s7,k6,k7 consumes G12,G14,G13,G15)
            gap_of = {("s", h): 2 * h for h in range(8)}
            gap_of.update({("k", h): 2 * h + 1 for h in range(8)})

        carry = []   # work woven into the next branch
        sN = {}
        for pos, (kind, h) in enumerate(schedule):
            g = pos if SCHED == "sskk" else gap_of[(kind, h)]
            if kind == "s":
                ctx_s, sfl = self_branch(h, units(g) + carry)

                # flush parts interleaved with the normalize halves they feed
                def mk_sn(i, h=h, ctx_s=ctx_s):
                    def f():
                        if i == 0:
                            sN[h] = snpool.tile([P, S // P, HD], F32,
                                                name=f"sN_{h}", tag="sN")
                        norm_part(h, ctx_s[i], i, sN[h])
                    return f

                carry = [sfl[0], sfl[1], (mk_sn(0), 0), sfl[2], sfl[3],
                         (mk_sn(1), 0)]
            else:
                ctx_k, kfl = knl_branch(h, units(g) + carry,
                                        split_last=(h == 7))

                tmp = {}

                def mk_k(i, h=h, ctx_k=ctx_k, tmp=tmp):
                    def f():
                        if i == 0:
                            tmp[0] = smallp.tile([P, S // P, HD], F32,
                                                 name=f"tK_{h}", tag="tK")
                        norm_part(h + 8, ctx_k[i], i, tmp[0])
                        oh = out_half[h // 4]
                        nc.vector.tensor_tensor(
                            out=oh[:, i * 4:i * 4 + 4, h % 4, :],
                            in0=tmp[0][:, i * 4:i * 4 + 4, :],
                            in1=sN[h][:, i * 4:i * 4 + 4, :], op=ALU.add)
                        if i == 1:
                            sN.pop(h)
                    return f

                carry = [kfl[0], kfl[1], (mk_k(0), 0), kfl[2], kfl[3],
                         (mk_k(1), 0)]
                if h % 2 == 1:
                    carry.insert(3, (lambda q=h // 2: out_dma(q, 0), 0))
                    carry.append((lambda q=h // 2: out_dma(q, 1), 0))
        for f, _ in carry:
            f()

    nc.finalize()
    return nc


def _get_nc():
    if "nc" not in _CACHE:
        _CACHE["nc"] = _build()
    return _CACHE["nc"]


def kernel(**inputs):
    inp = {k: np.asarray(v, dtype=np.float32) for k, v in inputs.items()}
    nc = _get_nc()

    E4 = ml_dtypes.float8_e4m3

    def f8(x):
        return np.ascontiguousarray(np.asarray(x).astype(E4))

    def hilo(x):
        hi = np.asarray(x).astype(E4)
        lo = (np.asarray(x) - hi.astype(np.float32)).astype(E4)
        return np.ascontiguousarray(hi), np.ascontiguousarray(lo)

    B = 4
    in_maps = []
    for core in range(8):
        b, hg = core // 2, core % 2
        sl = slice(hg * HG, (hg + 1) * HG)
        hsT = inp["hidden_states"][b].T
        ehsT = inp["encoder_hidden_states"][b].T
        hs_hi, hs_lo = hilo(hsT)
        ehs_hi, ehs_lo = hilo(ehsT)
        wqh, wql = hilo(inp["Wq"][:, sl] * WS)
        wkh, wkl = hilo(inp["Wk"][:, sl] * WS)
        m = {
            "hsT_hi": hs_hi, "hsT_lo": hs_lo,
            "ehsT_hi": ehs_hi, "ehsT_lo": ehs_lo,
            "wqk0": np.ascontiguousarray(np.concatenate(
                [wqh[:, 0:P], wkh[:, 0:P], wql[:, 0:P], wkl[:, 0:P]],
                axis=1)),
            "wqkr": np.ascontiguousarray(np.concatenate(
                [blk[:, jt * P:(jt + 1) * P]
                 for jt in range(1, 4)
                 for blk in (wqh, wkh, wql, wkl)], axis=1)),
        }
        for nm in ["v", "kq", "kk", "kv"]:
            hi, lo = hilo(inp[f"W{nm}"][:, sl] * WS)
            m[f"w{nm}_hi"] = hi
            m[f"w{nm}_lo"] = lo
        # packed small constants: QT/KT stay x32-scaled until the exp, so
        # the q/k/kq/kk biases scale to match
        def pjt(x, n):
            return np.asarray(x).reshape(n, P).T
        c0 = np.concatenate([
            pjt(inp["bq"][sl] * WS, 4), pjt(inp["bk"][sl] * WS, 4),
            pjt(inp["attention_mask"][b, 0, 0, :], 8),
            pjt(inp["encoder_attention_mask"][b, 0, 0, :], 4),
            pjt(inp["bkq"][sl] * WS, 4), pjt(inp["bkk"][sl] * WS, 4),
        ], axis=1).astype(np.float32)
        c1 = np.concatenate([
            np.broadcast_to(inp["bv"][sl], (P, HG)),
            np.broadcast_to(inp["bkv"][sl], (P, HG)),
        ], axis=1).astype(np.float32)
        m["c0"] = np.ascontiguousarray(c0)
        m["c1"] = np.ascontiguousarray(c1)
        in_maps.append(m)

    res = run_bass_kernel_spmd(nc, in_maps, core_ids=list(range(8)))

    outp = np.empty((B, S, H), np.float32)
    for core in range(8):
        b, hg = core // 2, core % 2
        outp[b, :, hg * HG:(hg + 1) * HG] = res.results[core]["out"]
    return outp


# revision 81
# speedup vs baseline: 1.0016x; 1.0002x over previous
"""Trainium2 Bass kernel for nn_BertSelfAttention_7962869367489.

Dual-branch (self + cross/"knowledge") BERT attention, B=4, S=1024, K=512,
H=1024, NH=16, HD=64, fp32.

Sharding: 8 cores = (batch b in 0..3) x (head-group hg in 0..1, 8 heads each).
All six projections are column-split by head-group; per-head attention is
entirely core-local; output columns are disjoint per core, so the gather is a
pure concatenation (no collectives).

Per-core pipeline:
  - All six projections run as fp8e4m3 DoubleRow matmuls (0.5 PE rows per
    output element, 2 contraction chunks per instruction) using a hi/lo
    residual split prepared on the host: X_hi=fp8(X), X_lo=fp8(X-X_hi) (raw,
    using fp8 subnormals), W'=32*W, W_hi=fp8(W'), W_lo=fp8(W'-W_hi). The
    three product sets hi*hi + lo_x*hi + hi*lo_w share one product scale (32)
    and accumulate in a single PSUM group; measured accuracy matches bf16.
    The x32 descale folds into the exp scale (QK side, /1024) and into the
    V bias-add multiply (1/32), so no extra DVE work.
  - Projections: QT/KT/KQT/KKT = W.T @ srcT in transposed orientation (bf16
    outs, carrying the x32 scale); Vaug/KVaug = srcT.T @ Wv in normal
    orientation with an augmented column of 2.0, so the ctx matmul also
    produces 2*softmax-denominator, folding the (ctx+kctx)*0.5 branch average
    into the normalization.
  - Per head h: scoresT[t,s] = K_h @ Q_h^T (bf16, contraction HD=64); exp on
    ACT with per-partition mask bias and 1/(8*1024) scale, written bf16;
    after the branch's exps, ctx[s,d|den] accumulates in PSUM in NORMAL
    orientation via lhsT = e-chunk [t,128s], rhs = Vaug_h [t,65] (bf16).
    Accumulation groups run sc-outer/kt-inner so each PSUM region hosts one
    group at a time (interleaved groups in one bank clobber each other).
  - Normalization + branch merge on DVE straight out of PSUM; output DMA'd
    in four head-pair quarters.
  - Remaining projections are split into ~0.64us (jt, sc) units and woven
    between attention branches, paced by PE time (~1420 cycles of filler
    per exp step) so the PE never outruns the ACT exp cadence; knowledge
    branch h runs right after self branch h, with the last four branches
    ordered s6 s7 k6 k7 (knl branches have the smaller exp deficit).
  - Unit order inside each weave gap follows the DMA arrival order: the
    weave is a static PE instruction order, so a unit emitted before its
    weights land head-of-line-blocks everything behind it. DMA chunks are
    ~0.25-0.5MB (smaller is HWDGE-issue-bound at ~650ns/DMA); all small
    constants ship as one host-packed array.
"""
import numpy as np
import ml_dtypes
from contextlib import ExitStack

import concourse.bacc as bacc
import concourse.tile as tile
import concourse.mybir as mybir
from concourse.bass_utils import run_bass_kernel_spmd

F32 = mybir.dt.float32
BF16 = mybir.dt.bfloat16
F8 = mybir.dt.float8e4
AF = mybir.ActivationFunctionType
ALU = mybir.AluOpType
DR = mybir.MatmulPerfMode.DoubleRow

P = 128
S = 1024        # query length
TKS = 1024      # self-branch key length
TKK = 512       # knowledge-branch key length
H = 1024        # model dim (projection contraction)
HG = 512        # per-core output width (8 heads x 64)
NHL = 8         # heads per core
HD = 64
HC = H // P     # 8 contraction chunks
WS = 32.0       # host-side weight prescale (fp8 range centering)
INV = 0.125     # 1/sqrt(64)
INVS = INV / (WS * WS)  # exp scale absorbing the x32 on both Q and K

_CACHE = {}


def _build():
    nc = bacc.Bacc(target_bir_lowering=False, debug=False)

    hsT_hi = nc.dram_tensor("hsT_hi", [H, S], F8, kind="ExternalInput")
    hsT_lo = nc.dram_tensor("hsT_lo", [H, S], F8, kind="ExternalInput")
    ehsT_hi = nc.dram_tensor("ehsT_hi", [H, TKK], F8, kind="ExternalInput")
    ehsT_lo = nc.dram_tensor("ehsT_lo", [H, TKK], F8, kind="ExternalInput")
    # host-packed [wq_hi jt0 | wk_hi jt0 | wq_lo jt0 | wk_lo jt0]: contiguous
    # 512B rows dodge the sub-512B-run DMA penalty on the startup-critical
    # first weight loads
    wqk0 = nc.dram_tensor("wqk0", [H, 4 * P], F8, kind="ExternalInput")
    # host-packed [wq_hi jt1-3 | wk_hi jt1-3 | wq_lo jt1-3 | wk_lo jt1-3]:
    # 1536B rows, one clean DMA for the rest of the q/k weights
    wqkr = nc.dram_tensor("wqkr", [H, 4 * 3 * P], F8, kind="ExternalInput")
    w_hi = {}
    w_lo = {}
    b_in = {}
    for nm in ["v", "kq", "kk", "kv"]:
        w_hi[nm] = nc.dram_tensor(f"w{nm}_hi", [H, HG], F8,
                                  kind="ExternalInput")
        w_lo[nm] = nc.dram_tensor(f"w{nm}_lo", [H, HG], F8,
                                  kind="ExternalInput")
    # host-packed small constants, one DMA slot: [bias_q(4) | bias_k(4) |
    # mask(8) | emask(4) | bias_kq(4) | bias_kk(4)] in (p, col) layout
    c0_in = nc.dram_tensor("c0", [P, 28], F32, kind="ExternalInput")
    # host-packed broadcast rows: [bias_v(512) | bias_kv(512)]
    c1_in = nc.dram_tensor("c1", [P, 2 * HG], F32, kind="ExternalInput")
    out = nc.dram_tensor("out", [S, HG], F32, kind="ExternalOutput")

    with tile.TileContext(nc) as tc, ExitStack() as ctx:
        const = ctx.enter_context(tc.tile_pool(name="const", bufs=1))
        persist = ctx.enter_context(tc.tile_pool(name="persist", bufs=1))
        epool = ctx.enter_context(tc.tile_pool(name="epool", bufs=17))
        smallp = ctx.enter_context(tc.tile_pool(name="smallp", bufs=2))
        snpool = ctx.enter_context(tc.tile_pool(name="snpool", bufs=3))
        psproj = ctx.enter_context(tc.tile_pool(name="psproj", bufs=2, space="PSUM"))
        psbig = ctx.enter_context(tc.tile_pool(name="psbig", bufs=2, space="PSUM"))
        psctx = ctx.enter_context(tc.tile_pool(name="psctx", bufs=2, space="PSUM"))

        # ---- constants (views into the two host-packed tiles) ----
        c0_sb = const.tile([P, 28], F32)
        c1_sb = const.tile([P, 2 * HG], F32)
        bias_col = {"q": c0_sb[:, 0:4], "k": c0_sb[:, 4:8],
                    "kq": c0_sb[:, 20:24], "kk": c0_sb[:, 24:28]}
        mask_sb = c0_sb[:, 8:16]
        emask_sb = c0_sb[:, 16:20]
        bias_row = {"v": c1_sb[:, 0:HG], "kv": c1_sb[:, HG:2 * HG]}
        twos = const.tile([P, 1], F32)
        nc.vector.memset(twos, 2.0)

        def load_consts_early():
            nc.sync.dma_start(out=c0_sb, in_=c0_in.ap())

        def load_consts_late():
            nc.sync.dma_start(out=c1_sb, in_=c1_in.ap())

        # ---- persistent activations ----
        QT = persist.tile([P, 4, S], BF16)       # [j%128, jt, s] (x32 scaled)
        KT = persist.tile([P, 4, TKS], BF16)
        KQT = persist.tile([P, 4, S], BF16)
        KKT = persist.tile([P, 4, TKK], BF16)
        Vaug = persist.tile([P, TKS // P, NHL, HD + 1], BF16)   # [t%128, tt, h, d|2]
        KVaug = persist.tile([P, TKK // P, NHL, HD + 1], BF16)
        hsT_sb = {0: persist.tile([P, HC, S], F8, name="hsT_hi_sb"),
                  1: persist.tile([P, HC, S], F8, name="hsT_lo_sb")}
        ehsT_sb = {0: persist.tile([P, HC, TKK], F8, name="ehsT_hi_sb"),
                   1: persist.tile([P, HC, TKK], F8, name="ehsT_lo_sb")}
        wsb_hi = {}
        wsb_lo = {}
        for nm in ["v", "kq", "kk", "kv"]:
            wsb_hi[nm] = persist.tile([P, HC, HG], F8, name=f"wh_{nm}")
            wsb_lo[nm] = persist.tile([P, HC, HG], F8, name=f"wl_{nm}")
        wqk0_sb = persist.tile([P, HC, 4 * P], F8)
        wqkr_sb = persist.tile([P, HC, 4 * 3 * P], F8)
        # output staging in two head-halves; DMA'd in four head-pair quarters
        out_half = [persist.tile([P, S // P, 4, HD], F32, name=f"out_half{i}",
                                 tag=f"out_half{i}") for i in range(2)]

        # q/k weight operand lookup: (tile, hi col base, lo col base) per jt.
        # wqkr is packed per-jt: [jt block][qh|kh|ql|kl] so each jt's weights
        # load as one contiguous-row DMA in deadline order.
        def qk_cols(nm, jt):
            off = 0 if nm == "q" else 1
            if jt == 0:
                return wqk0_sb, off * P, (2 + off) * P
            base = (jt - 1) * 4 * P
            return wqkr_sb, base + off * P, base + (2 + off) * P

        # ---- input DMAs (sync/HWDGE queue), ordered so the prelude's
        # dependencies (wqk0, hsT_hi, then hsT_lo) land first ----
        def load_rows(dst, src, half, rows, cols):
            nc.sync.dma_start(
                out=dst[:, half * (rows // 2):(half + 1) * (rows // 2), :],
                in_=src[half * (rows * P // 2):(half + 1) * (rows * P // 2), :]
                .rearrange("(hc p) s -> p hc s", p=P))

        def load_w(nm, which):
            src = w_hi[nm] if which == 0 else w_lo[nm]
            dst = wsb_hi[nm] if which == 0 else wsb_lo[nm]
            nc.sync.dma_start(
                out=dst, in_=src.ap().rearrange("(hc p) j -> p hc j", p=P))

        def load_wqk0(hc0, hcn):
            nc.sync.dma_start(
                out=wqk0_sb[:, hc0:hc0 + hcn, :],
                in_=wqk0[hc0 * P:(hc0 + hcn) * P, :].rearrange(
                    "(hc p) j -> p hc j", p=P))

        def load_hsT(which, hc0, hcn):
            src = hsT_hi if which == 0 else hsT_lo
            nc.sync.dma_start(
                out=hsT_sb[which][:, hc0:hc0 + hcn, :],
                in_=src[hc0 * P:(hc0 + hcn) * P, :].rearrange(
                    "(hc p) s -> p hc s", p=P))

        # startup-critical loads in strict first-use order: the prelude's
        # set-A matmuls need wqk0(hi cols) + hsT_hi (first two chunks feed
        # single-hc fp8 matmuls so the PE starts before the pair lands);
        # set-C needs wqk0(lo cols); set-B needs hsT_lo. Then weights in
        # weave-consumption order: wv (G0's V units), wkq/wkk+ehsT (knl0),
        # wkv (kv units), wqkr (q/k jt1-3).
        # 2-hc (0.25MB) chunks: smaller chunks are HWDGE-issue-bound (625ns
        # hold per DMA), larger ones delay the first matmul
        load_wqk0(0, 2)
        load_hsT(0, 0, 2)
        for hc2 in (2, 4, 6):
            load_wqk0(hc2, 2)
            load_hsT(0, hc2, 2)
        load_consts_early()
        # hsT_lo in fine chunks: the prelude's set-B hc-pairs stream as they
        # land instead of waiting for one big transfer
        load_hsT(1, 0, 2)
        load_hsT(1, 2, 2)
        load_hsT(1, 4, 2)
        load_hsT(1, 6, 2)
        load_w("v", 0)
        load_w("v", 1)
        load_consts_late()
        load_w("kq", 0)
        load_w("kq", 1)
        load_rows(ehsT_sb[0], ehsT_hi, 0, HC, TKK)
        load_rows(ehsT_sb[0], ehsT_hi, 1, HC, TKK)
        load_w("kk", 0)
        load_w("kk", 1)
        load_rows(ehsT_sb[1], ehsT_lo, 0, HC, TKK)
        load_rows(ehsT_sb[1], ehsT_lo, 1, HC, TKK)
        load_w("kv", 0)
        load_w("kv", 1)
        for jt in range(3):
            nc.sync.dma_start(
                out=wqkr_sb[:, :, jt * 4 * P:(jt + 1) * 4 * P],
                in_=wqkr[:, jt * 4 * P:(jt + 1) * 4 * P].rearrange(
                    "(hc p) j -> p hc j", p=P))

        # ---- projection emitters (fp8 hi/lo DoubleRow, single PSUM group:
        # sets (w_hi,x_hi), (w_hi,x_lo), (w_lo,x_hi), 4 hc-pairs each) ----
        def proj_t_unit(nm, dst, srcT, jt, sc, w=256):
            if nm in ("q", "k"):
                wt_h, ch, cl = qk_cols(nm, jt)
                wt_l = wt_h
            else:
                wt_h, wt_l = wsb_hi[nm], wsb_lo[nm]
                ch = cl = jt * P
            ps = psproj.tile([P, 512], F32, name="psj", tag="psj")
            mm = 0
            for wt, c0, xs in ((wt_h, ch, 0), (wt_h, ch, 1), (wt_l, cl, 0)):
                for hp in range(4):
                    nc.tensor.matmul(
                        ps[:, 0:w],
                        lhsT=wt[:, 2 * hp:2 * hp + 2, c0:c0 + P],
                        rhs=srcT[xs][:, 2 * hp:2 * hp + 2,
                                     sc * w:(sc + 1) * w],
                        start=(mm == 0), stop=(mm == 11), perf_mode=DR)
                    mm += 1
            nc.vector.tensor_scalar_add(
                dst[:, jt, sc * w:(sc + 1) * w], ps[:, 0:w],
                bias_col[nm][:, jt:jt + 1])

        def proj_v_part(nm, dst, srcT, tt, h0, nh):
            # V-projection for a head subrange (nh <= 4 keeps the DoubleRow
            # moving free dim within 512); the 1/WS descale rides the
            # existing bias-add multiply
            wt_h = wsb_hi["v" if nm == "v" else "kv"]
            wt_l = wsb_lo["v" if nm == "v" else "kv"]
            ps = psproj.tile([P, 512], F32, name=f"psv{tt}_{h0}", tag="psj")
            mm = 0
            for wt, xs in ((wt_h, 0), (wt_h, 1), (wt_l, 0)):
                for hp in range(4):
                    nc.tensor.matmul(
                        ps[:, 0:nh * HD],
                        lhsT=srcT[xs][:, 2 * hp:2 * hp + 2,
                                      tt * P:(tt + 1) * P],
                        rhs=wt[:, 2 * hp:2 * hp + 2,
                               h0 * HD:(h0 + nh) * HD],
                        start=(mm == 0), stop=(mm == 11), perf_mode=DR)
                    mm += 1
            nc.vector.scalar_tensor_tensor(
                out=dst[:, tt, h0:h0 + nh, 0:HD],
                in0=ps[:, 0:nh * HD].rearrange("p (h d) -> p h d", h=nh),
                scalar=1.0 / WS,
                in1=bias_row[nm].rearrange(
                    "p (h d) -> p h d", h=NHL)[:, h0:h0 + nh, :],
                op0=ALU.mult, op1=ALU.add)
            nc.vector.tensor_copy(
                dst[:, tt, h0:h0 + nh, HD:HD + 1],
                twos.unsqueeze(1).broadcast_to([P, nh, 1]))

        # ---- prelude: Q/K jt0 in eight 256-wide groups, two phases of four
        # concurrent groups (two in the idle scores-psum slots); matmuls woven
        # in DMA-arrival order (set A needs only hi operands, set B needs
        # hsT_lo which lands last) so the PE starts and p-state-ramps early
        def prelude_phase(groups, single_start=False):
            """Q/K jt0 groups [(nm, sc, pool, tag)]: concurrent 256-wide
            accumulation groups, one PSUM bank each (psbig's banks are idle
            until the first scores matmul, so the first phase borrows them
            for two extra groups). Sets A (w_hi,x_hi) and C (w_lo,x_hi) are
            interleaved per hc-pair -- both consume the same arriving hsT_hi
            chunks, so interleaving doubles the ready work per DMA landing;
            set B (w_hi,x_lo) streams with the hsT_lo chunks. single_start
            emits set A's first hc pair as two plain single-hc fp8 matmuls
            so the PE starts as soon as (wqk0 hc0, hsT_hi hc0) land."""
            pre_ps = dict(groups)
            keys = [k for k, _ in groups]
            mm = {k: 0 for k in pre_ps}
            w = 256
            nmm = 13 if single_start else 12

            def emit(nm, sc, c0, xs, hp, single=None):
                k = (nm, sc)
                if single is None:
                    lhsT = wqk0_sb[:, 2 * hp:2 * hp + 2, c0:c0 + P]
                    rhs = hsT_sb[xs][:, 2 * hp:2 * hp + 2,
                                     sc * w:(sc + 1) * w]
                    pm = DR
                else:
                    lhsT = wqk0_sb[:, single, c0:c0 + P]
                    rhs = hsT_sb[xs][:, single, sc * w:(sc + 1) * w]
                    pm = None
                nc.tensor.matmul(pre_ps[k], lhsT=lhsT, rhs=rhs,
                                 start=(mm[k] == 0), stop=(mm[k] == nmm - 1),
                                 perf_mode=pm)
                mm[k] += 1

            hps_a = [0.5, 1, 2, 3] if single_start else [0, 1, 2, 3]
            for hp in hps_a:
                for nm, sc in keys:
                    ch, cl = qk_cols(nm, 0)[1:]
                    if hp == 0.5:
                        emit(nm, sc, ch, 0, 0, single=0)
                        emit(nm, sc, ch, 0, 0, single=1)
                        emit(nm, sc, cl, 0, 0)
                    else:
                        emit(nm, sc, ch, 0, hp)
                        if hp >= 1:
                            emit(nm, sc, cl, 0, hp)
            if not single_start:
                for nm, sc in keys:
                    ch, cl = qk_cols(nm, 0)[1:]
                    emit(nm, sc, cl, 0, 0)
            for hp in range(4):
                for nm, sc in keys:
                    ch, cl = qk_cols(nm, 0)[1:]
                    emit(nm, sc, ch, 1, hp)
            # bias-adds ordered so the first scores matmul's inputs (QT
            # s-half 0 and KT key-chunk 0) complete first
            for nm, sc in keys:
                dst = QT if nm == "q" else KT
                nc.vector.tensor_scalar_add(
                    dst[:, 0, sc * w:(sc + 1) * w],
                    pre_ps[(nm, sc)], bias_col[nm][:, 0:1])

        # all eight jt0 groups run in ONE phase: the two psbig tiles span two
        # banks each, so each hosts two groups in disjoint banks (cols 0-255
        # and 512-767); no group waits on another's bias-add
        pre_t1 = psproj.tile([P, 512], F32, name="pre_a", tag="psj")
        pre_t2 = psproj.tile([P, 512], F32, name="pre_b", tag="psj")
        pre_t3 = psctx.tile([P, 512], F32, name="pre_c", tag="ctxps")
        pre_t4 = psctx.tile([P, 512], F32, name="pre_d", tag="ctxps")
        pre_t5 = psbig.tile([P, S], F32, name="pre_e", tag="big")
        pre_t6 = psbig.tile([P, S], F32, name="pre_f", tag="big")
        prelude_phase([
            (("q", 0), pre_t1[:, 0:256]), (("k", 0), pre_t2[:, 0:256]),
            (("q", 1), pre_t3[:, 0:256]), (("k", 1), pre_t4[:, 0:256]),
            (("q", 2), pre_t5[:, 0:256]), (("k", 2), pre_t6[:, 0:256]),
            (("q", 3), pre_t5[:, 512:768]), (("k", 3), pre_t6[:, 512:768]),
        ], single_start=True)

        # ---- filler units: remaining projections, drained between branches
        def fill_unit(u):
            nm = u[0]
            if nm == "v":
                proj_v_part("v", Vaug, hsT_sb, u[1], u[2], u[3])
            elif nm == "kv":
                proj_v_part("kv", KVaug, ehsT_sb, u[1], u[2], u[3])
            elif nm == "kk":
                proj_t_unit("kk", KKT, ehsT_sb, u[1], u[2])
            elif nm == "q":
                proj_t_unit("q", QT, hsT_sb, u[1], u[2])
            elif nm == "k":
                proj_t_unit("k", KT, hsT_sb, u[1], u[2])
            elif nm == "kq":
                proj_t_unit("kq", KQT, hsT_sb, u[1], u[2])

        def sc_units(nm, jt, nsc):
            return [(nm, jt, sc) for sc in range(nsc)]

        # gap g -> units woven into the branch at schedule position g.
        # Units are woven INSIDE that branch's kt loop (paced across the kt
        # steps) so ready filler work sits between the ACT-dependent
        # scores/ctx matmuls in the PE queue; leftovers drain right after the
        # branch, which still meets every deadline. Schedule is
        # s0 s1 k0 k1 s2 s3 k2 k3 s4 s5 k4 k5 s6 s7 k6 k7; deadlines:
        # q/k jt before self(2jt); kq/kk jt before knl(2jt); V head h before
        # the branch after self(h) (its sflush); KV head h before the branch
        # after knl(h). Early gaps follow the DMA arrival order (wv, then
        # wkq/wkk/ehsT, then wkv, then wqkr).
        SCHED = "sk"
        # Unit order within each gap follows DMA arrival order (the weave is
        # a static PE instruction order, so a unit emitted before its weights
        # land head-of-line-blocks everything behind it).
        GAPS_SK = [
            # old (s,k)x6 + s6,s7,k6,k7 schedule; gap g consumed by branch at
            # position g
            [("v", t, 0, 4) for t in range(8)]
            + sc_units("kq", 0, 4) + sc_units("kk", 0, 2),   # G0 (s0)
            [("v", t, 4, 2) for t in range(8)],              # G1 (k0)
            [("kv", t, 0, 4) for t in range(4)]
            + sc_units("q", 1, 4),                           # G2 (s1)
            sc_units("k", 1, 4),                             # G3 (k1)
            sc_units("kq", 1, 4) + sc_units("kk", 1, 2),     # G4 (s2)
            sc_units("q", 2, 4)[:2],                         # G5 (k2)
            sc_units("q", 2, 4)[2:],                         # G6 (s3)
            sc_units("k", 2, 4),                             # G7 (k3)
            sc_units("kq", 2, 4) + sc_units("kk", 2, 2),     # G8 (s4)
            [("kv", t, 4, 2) for t in range(4)],             # G9 (k4)
            sc_units("q", 3, 4),                             # G10 (s5)
            sc_units("k", 3, 4),                             # G11 (k5)
            sc_units("kq", 3, 4)
            + [("v", t, 6, 1) for t in range(8)],            # G12 (s6)
            [("kv", t, 6, 1) for t in range(4)],             # G13 (k6)
            sc_units("kk", 3, 2)
            + [("v", t, 7, 1) for t in range(8)],            # G14 (s7)
            [("kv", t, 7, 1) for t in range(4)],             # G15 (k7)
        ]
        GAPS = [
            # G0 (s0): V heads 0,1 (wv is the first weight to land), then
            # kq jt0 (wkq lands mid-s0; kq units sit at the late slots)
            [("v", t, 0, 1) for t in range(8)]
            + [("v", t, 1, 1) for t in range(8)]
            + sc_units("kq", 0, 4),
            # G1 (s1): V h2-3 first (wv resident), kk jt0 once ehsT+wkk have
            # landed mid-window
            [("v", t, 2, 2) for t in range(8)] + sc_units("kk", 0, 2),
            # G2 (k0): KV h0,h1 (wkv lands just before k0), then q jt1
            # (wqkr jt1 lands at k0's start)
            [("kv", t, 0, 1) for t in range(4)]
            + [("kv", t, 1, 1) for t in range(4)]
            + sc_units("q", 1, 4),
            sc_units("k", 1, 4),                           # G3 (k1): self2 needs jt1
            sc_units("kq", 1, 4) + sc_units("kk", 1, 2),   # G4 (s2): knl2 needs jt1
            [("v", t, 4, 2) for t in range(8)]
            + [("kv", t, 2, 2) for t in range(4)],         # G5 (s3)
            sc_units("q", 2, 4),                           # G6 (k2)
            sc_units("k", 2, 4),                           # G7 (k3): self4 needs jt2
            sc_units("kq", 2, 4) + sc_units("kk", 2, 2),   # G8 (s4): knl4 needs jt2
            [("kv", t, 4, 2) for t in range(4)]
            + sc_units("q", 3, 4),                         # G9 (s5)
            sc_units("k", 3, 4),                           # G10 (k4): self6 needs jt3
            sc_units("kq", 3, 2),                          # G11 (k5)
            [("v", t, 6, 1) for t in range(8)]
            + sc_units("kk", 3, 2)
            + [("v", t, 7, 1) for t in range(8)],          # G12 (s6)
            sc_units("kq", 3, 4)[2:]
            + [("kv", t, 6, 1) for t in range(4)]
            + [("kv", t, 7, 1) for t in range(4)],         # G13 (s7): knl6 needs jt3
            [],                                            # G14 (k6)
            [],                                            # G15 (k7)
        ]

        # ---- attention branches ----
        # `weave` is a list of (callable, pe_cycles) items (filler units,
        # previous-branch ctx flush parts, normalize/merge closures) emitted
        # across the kt steps paced by PE time: each exp step gets ~WEAVE_TGT
        # cycles of woven PE work, matching the ACT exp cadence so neither
        # engine runs dry.
        WEAVE_TGT = 1420.0  # (1038ns exp - 427ns scores) * 2.4 cycles/ns

        def head_branch(h, kt_mat, q_mat, vaug, n_keys, msk, weave,
                        split_last=False):
            base = (h % 2) * HD
            jt = h // 2
            nkt = n_keys // P
            ctxA = psctx.tile([P, 4, HD + 1], F32, name=f"cA_{h}_{n_keys}",
                              tag="ctxps")
            ctxB = psctx.tile([P, 4, HD + 1], F32, name=f"cB_{h}_{n_keys}",
                              tag="ctxps")

            nw = len(weave)
            slots = []
            cum = 0.0
            for fn, cost in weave:
                slots.append(min(nkt - 1, int(cum / WEAVE_TGT)))
                cum += cost
            e_ts = []
            for kt in range(nkt):
                st_ps = psbig.tile([P, S], F32, name=f"st_{h}_{kt}", tag="big")
                for sc2 in range(S // 512):
                    nc.tensor.matmul(
                        st_ps[:, sc2 * 512:(sc2 + 1) * 512],
                        lhsT=kt_mat[base:base + HD, jt, kt * P:(kt + 1) * P],
                        rhs=q_mat[base:base + HD, jt, sc2 * 512:(sc2 + 1) * 512],
                        start=True, stop=True)
                e_t = epool.tile([P, S], BF16, name=f"e_{h}_{kt}", tag="e")
                if split_last and kt >= nkt - 2:
                    # halve the last exp so the s<512 ctx groups (and the
                    # final normalize/merge/DMA chain) start half an exp early
                    for eh in range(2):
                        nc.scalar.activation(
                            e_t[:, eh * 512:(eh + 1) * 512],
                            st_ps[:, eh * 512:(eh + 1) * 512], AF.Exp,
                            bias=msk[:, kt:kt + 1], scale=INVS)
                else:
                    nc.scalar.activation(e_t, st_ps, AF.Exp,
                                         bias=msk[:, kt:kt + 1], scale=INVS)
                e_ts.append(e_t)
                for j in range(nw):
                    if slots[j] == kt:
                        weave[j][0]()

            def flush_part(scs):
                # sc-outer / kt-inner: one open accumulation group per PSUM
                # region at a time (interleaved groups corrupt each other)
                for sc in scs:
                    t = ctxA if sc < 4 else ctxB
                    for kt in range(nkt):
                        nc.tensor.matmul(
                            t[:, sc % 4, :],
                            lhsT=e_ts[kt][:, sc * P:(sc + 1) * P],
                            rhs=vaug[:, kt, h, :],
                            start=(kt == 0), stop=(kt == nkt - 1))

            flush_parts = [(lambda scs=(sc0, sc0 + 1): flush_part(scs),
                            2 * nkt * 65)
                           for sc0 in range(0, S // P, 2)]
            return (ctxA, ctxB), flush_parts

        def self_branch(h, weave):
            return head_branch(h, KT, QT, Vaug, TKS, mask_sb, weave)

        def knl_branch(h, weave, split_last=False):
            return head_branch(h, KKT, KQT, KVaug, TKK, emask_sb, weave,
                               split_last)

        def norm_part(h, t, i, dst):
            # dst[:, 4i:4i+4, :] = ctx-tile / (2*denominator)
            rb = smallp.tile([P, 4, 1], F32, name=f"rb_{h}_{i}", tag="rb",
                             bufs=4)
            nc.vector.reciprocal(rb, t[:, :, HD:HD + 1])
            nc.vector.tensor_tensor(
                out=dst[:, i * 4:i * 4 + 4, :], in0=t[:, :, 0:HD],
                in1=rb.broadcast_to([P, 4, HD]), op=ALU.mult)

        def out_dma(q, half):
            # head-pair quarter q: heads 2q, 2q+1 -> out columns [128q, 128q+128),
            # s-half `half` -> rows [512*half, 512*half+512)
            oh = out_half[q // 2]
            hp = (2 * q) % 4
            nc.sync.dma_start(
                out=out.ap()[half * 512:(half + 1) * 512,
                             q * P:(q + 1) * P].rearrange(
                    "(sc p) j -> p sc j", p=P),
                in_=oh[:, half * 4:(half + 1) * 4, hp:hp + 2, :].rearrange(
                    "p sc h d -> p sc (h d)"))

        # ---- main schedule: self(h) then knl(h); each branch weaves in the
        # previous branch's ctx flush + normalize/merge plus this gap's
        # projection units, so the ACT-bound exp chain is always overlapped
        # with ready PE work ----
        def unit_cost(u):
            if u[0] in ("v", "kv"):
                return 12 * u[3] * HD // 2
            return 12 * P
        def units(g):
            return [(lambda u=u: fill_unit(u), unit_cost(u)) for u in GAPS[g]]

        # self-self-knl-knl pairs: defers each knowledge branch's operand
        # needs (wkq/wkk/ehsT) a full window behind the DMA stream, and ends
        # on knl branches whose smaller exp-vs-PE deficit shrinks tail idle.
        # The "sk" variant is the baseline's (s,k) pairing with the same
        # knl-heavy tail.
        schedule = []
        if SCHED == "sskk":
            for hp2 in range(4):
                schedule += [("s", 2 * hp2), ("s", 2 * hp2 + 1),
                             ("k", 2 * hp2), ("k", 2 * hp2 + 1)]
        else:
            GAPS = GAPS_SK
            for h in range(6):
                schedule += [("s", h), ("k", h)]
            schedule += [("s", 6), ("s", 7), ("k", 6), ("k", 7)]
            # gap index by head as in the baseline: self h -> 2h, knl h ->
            # 2h+1 (so the tail order s6,s7,k6,k7 consumes G12,G14,G13,G15)
            gap_of = {("s", h): 2 * h for h in range(8)}
            gap_of.update({("k", h): 2 * h + 1 for h in range(8)})

        carry = []   # work woven into the next branch
        sN = {}
        for pos, (kind, h) in enumerate(schedule):
            g = pos if SCHED == "sskk" else gap_of[(kind, h)]
            if kind == "s":
                ctx_s, sfl = self_branch(h, units(g) + carry)

                # flush parts interleaved with the normalize halves they feed
                def mk_sn(i, h=h, ctx_s=ctx_s):
                    def f():
                        if i == 0:
                            sN[h] = snpool.tile([P, S // P, HD], F32,
                                                name=f"sN_{h}", tag="sN")
                        norm_part(h, ctx_s[i], i, sN[h])
                    return f

                carry = [sfl[0], sfl[1], (mk_sn(0), 0), sfl[2], sfl[3],
                         (mk_sn(1), 0)]
            else:
                ctx_k, kfl = knl_branch(h, units(g) + carry,
                                        split_last=(h == 7))

                tmp = {}

                def mk_k(i, h=h, ctx_k=ctx_k, tmp=tmp):
                    def f():
                        if i == 0:
                            tmp[0] = smallp.tile([P, S // P, HD], F32,
                                                 name=f"tK_{h}", tag="tK")
                        norm_part(h + 8, ctx_k[i], i, tmp[0])
                        oh = out_half[h // 4]
                        nc.vector.tensor_tensor(
                            out=oh[:, i * 4:i * 4 + 4, h % 4, :],
                            in0=tmp[0][:, i * 4:i * 4 + 4, :],
                            in1=sN[h][:, i * 4:i * 4 + 4, :], op=ALU.add)
                        if i == 1:
                            sN.pop(h)
                    return f

                carry = [kfl[0], kfl[1], (mk_k(0), 0), kfl[2], kfl[3],
                         (mk_k(1), 0)]
                if h % 2 == 1:
                    carry.insert(3, (lambda q=h // 2: out_dma(q, 0), 0))
                    carry.append((lambda q=h // 2: out_dma(q, 1), 0))
        for f, _ in carry:
            f()

    nc.finalize()
    return nc


def _get_nc():
    if "nc" not in _CACHE:
        _CACHE["nc"] = _build()
    return _CACHE["nc"]


def kernel(**inputs):
    inp = {k: np.asarray(v, dtype=np.float32) for k, v in inputs.items()}
    nc = _get_nc()

    E4 = ml_dtypes.float8_e4m3

    def f8(x):
        return np.ascontiguousarray(np.asarray(x).astype(E4))

    def hilo(x):
        hi = np.asarray(x).astype(E4)
        lo = (np.asarray(x) - hi.astype(np.float32)).astype(E4)
        return np.ascontiguousarray(hi), np.ascontiguousarray(lo)

    B = 4
    in_maps = []
    for core in range(8):
        b, hg = core // 2, core % 2
        sl = slice(hg * HG, (hg + 1) * HG)
        hsT = inp["hidden_states"][b].T
        ehsT = inp["encoder_hidden_states"][b].T
        hs_hi, hs_lo = hilo(hsT)
        ehs_hi, ehs_lo = hilo(ehsT)
        wqh, wql = hilo(inp["Wq"][:, sl] * WS)
        wkh, wkl = hilo(inp["Wk"][:, sl] * WS)
        m = {
            "hsT_hi": hs_hi, "hsT_lo": hs_lo,
            "ehsT_hi": ehs_hi, "ehsT_lo": ehs_lo,
            "wqk0": np.ascontiguousarray(np.concatenate(
                [wqh[:, 0:P], wkh[:, 0:P], wql[:, 0:P], wkl[:, 0:P]],
                axis=1)),
            "wqkr": np.ascontiguousarray(np.concatenate(
                [blk[:, jt * P:(jt + 1) * P]
                 for jt in range(1, 4)
                 for blk in (wqh, wkh, wql, wkl)], axis=1)),
        }
        for nm in ["v", "kq", "kk", "kv"]:
            hi, lo = hilo(inp[f"W{nm}"][:, sl] * WS)
            m[f"w{nm}_hi"] = hi
            m[f"w{nm}_lo"] = lo
        # packed small constants: QT/KT stay x32-scaled until the exp, so
        # the q/k/kq/kk biases scale to match
        def pjt(x, n):
            return np.asarray(x).reshape(n, P).T
        c0 = np.concatenate([
            pjt(inp["bq"][sl] * WS, 4), pjt(inp["bk"][sl] * WS, 4),
            pjt(inp["attention_mask"][b, 0, 0, :], 8),
            pjt(inp["encoder_attention_mask"][b, 0, 0, :], 4),
            pjt(inp["bkq"][sl] * WS, 4), pjt(inp["bkk"][sl] * WS, 4),
        ], axis=1).astype(np.float32)
        c1 = np.concatenate([
            np.broadcast_to(inp["bv"][sl], (P, HG)),
            np.broadcast_to(inp["bkv"][sl], (P, HG)),
        ], axis=1).astype(np.float32)
        m["c0"] = np.ascontiguousarray(c0)
        m["c1"] = np.ascontiguousarray(c1)
        in_maps.append(m)

    res = run_bass_kernel_spmd(nc, in_maps, core_ids=list(range(8)))

    outp = np.empty((B, S, H), np.float32)
    for core in range(8):
        b, hg = core // 2, core % 2
        outp[b, :, hg * HG:(hg + 1) * HG] = res.results[core]["out"]
    return outp
